# revision 1
# baseline (speedup 1.0000x reference)
"""ClusterGCN (3-layer GCN, sum-aggregation) on 8 Trainium2 NeuronCores.

Strategy (hardcoded for B=2, N=50000, F=H=128, E=800000, 8 cores):
  - core c: batch b=c//4, destination shard q=c%4 (12500 nodes each).
  - Reassociate each layer: A @ (h @ W) == (A @ h) @ W, so every layer is
    gather-aggregate (segment-sum over edges) followed by a dense 128x128
    matmul. Aggregation output lives as agg_T[f, d] (features on partitions).
  - Per-edge gathers use the SWDGE dma_gather custom instruction (int16
    indices, max 1024 per call). Indices only reach 32767 rows, so each edge
    stream is split into a low (src < 25000) and high (src >= 25000) bucket
    gathered from offset views of the table.
  - Segment-sum on the tensor engine: edges are grouped per 16-destination
    window into 128-slot chunks; chunk x onehot(dest_rel) matmuls accumulate
    into a PSUM tile of 128 destinations. Window overflow edges go to
    per-tile overflow chunks gathered in batched calls across tile groups.
  - BatchNorm is training-mode over all B*N rows: per-core bn_stats/bn_aggr,
    then an 8-core AllReduce of (mean, E[x^2]).
  - After BN+ReLU the shard rows are transposed back to row-major and
    AllGathered (groups of 4 cores = one batch) into the next gather table.
"""

import math

import numpy as np

# ---------------------------------------------------------------- config

P = 128
FEAT = 128
WIN = 16
WPT = P // WIN  # windows per tile (8); one window call = WPT*128 = 1024 idx


class Cfg:
    def __init__(self, n_nodes=50000, shard=12500, batch=2, eps=1e-5):
        assert shard * 4 == n_nodes
        self.N = n_nodes
        self.SHARD = shard
        self.BATCH = batch
        self.HALF = n_nodes // 2
        assert self.HALF <= 32767
        self.TILES = math.ceil(shard / P)
        self.VALID_LAST = shard - (self.TILES - 1) * P
        self.EPS = eps
        # debug toggles
        self.LAYERS = 3
        self.USE_AR = True
        self.USE_AG = True


# ---------------------------------------------------------------- host schedule


def _shard_schedule(cfg, row, col, q):
    """Per-shard edge schedule: window slots + overflow lists.

    Returns (win_idx [NW,2,128] int16, win_drel [NW,2,128] f32,
             ov: dict[(tile, bucket)] -> (idx16 1d, drel 1d))."""
    base = q * cfg.SHARD
    m = (col >= base) & (col < base + cfg.SHARD)
    r = row[m].astype(np.int64)
    c = (col[m] - base).astype(np.int64)
    wg = c // WIN
    bkt = (r >= cfg.HALF).astype(np.int64)
    key = wg * 2 + bkt
    order = np.argsort(key, kind="stable")
    r, c, wg, bkt, key = r[order], c[order], wg[order], bkt[order], key[order]
    n = len(key)
    NW = cfg.TILES * WPT

    if n == 0:
        return (
            np.zeros((NW, 2, P), np.int16),
            np.full((NW, 2, P), 255.0, np.float32),
            {},
        )

    newg = np.empty(n, bool)
    newg[0] = True
    newg[1:] = key[1:] != key[:-1]
    gstart = np.flatnonzero(newg)
    counts = np.diff(np.append(gstart, n))
    starts = np.repeat(gstart, counts)
    pos = np.arange(n) - starts
    idx16 = np.where(bkt == 1, r - cfg.HALF, r).astype(np.int16)

    inw = pos < P
    win_idx = np.zeros((NW, 2, P), np.int16)
    win_drel = np.full((NW, 2, P), 255.0, np.float32)
    win_idx[wg[inw], bkt[inw], pos[inw]] = idx16[inw]
    win_drel[wg[inw], bkt[inw], pos[inw]] = (c[inw] - wg[inw] * WIN).astype(np.float32)

    ov = {}
    ow = ~inw
    if ow.any():
        t_ov = wg[ow] // WPT
        b_ov = bkt[ow]
        i_ov = idx16[ow]
        d_ov = (c[ow] - t_ov * P).astype(np.float32)
        okey = t_ov * 2 + b_ov
        oorder = np.argsort(okey, kind="stable")
        t_ov, b_ov, i_ov, d_ov, okey = (
            t_ov[oorder],
            b_ov[oorder],
            i_ov[oorder],
            d_ov[oorder],
            okey[oorder],
        )
        bounds = np.flatnonzero(np.append(True, okey[1:] != okey[:-1]))
        bounds = np.append(bounds, len(okey))
        for j in range(len(bounds) - 1):
            s, e = bounds[j], bounds[j + 1]
            ov[(int(t_ov[s]), int(b_ov[s]))] = (i_ov[s:e], d_ov[s:e])
    return win_idx, win_drel, ov


def _wrap16(stream):
    """[n] idx stream -> [128, n/16] wrapped col-major, replicated x8."""
    return np.tile(stream.reshape(-1, 16).T, (8, 1))


def build_schedule(cfg, edge_index):
    """Build gather-index / dest-rel input tensors for the 4 shards.

    Layout (layer-invariant, loaded once):
      win_idx  [128, TILES*128] i16 : tile t -> cols [t*128, t*128+64) = lo
               window stream (8 chunks), [+64, +128) = hi stream.
      win_drel [128, TILES*16] f32  : tile t -> cols [t*16+w] lo, [t*16+8+w] hi.
      ov_idx   [128, NOVG*OG*OVC*8*2] i16 : group og -> lo block then hi block.
      ov_drel  [128, NOVG*OG*OVC*2] f32   : group og -> lo cols then hi cols.

    Returns (per_shard list of dicts, OVC, OG)."""
    row = np.asarray(edge_index[0])
    col = np.asarray(edge_index[1])
    shards = [_shard_schedule(cfg, row, col, q) for q in range(4)]

    ovc = 0
    for _, _, ov in shards:
        for (t, b), (i1, _) in ov.items():
            ovc = max(ovc, math.ceil(len(i1) / P))
    OVC = ovc
    OG = max(1, WPT // OVC) if OVC else 1
    NOVG = math.ceil(cfg.TILES / OG)

    per_shard = []
    for win_idx, win_drel, ov in shards:
        wi = np.zeros((128, cfg.TILES * P), np.int16)
        wd = np.full((128, cfg.TILES * WIN), 255.0, np.float32)
        for t in range(cfg.TILES):
            for b in (0, 1):
                stream = np.concatenate(
                    [win_idx[t * WPT + w, b] for w in range(WPT)]
                )
                wi[:, t * P + b * 64 : t * P + (b + 1) * 64] = _wrap16(stream)
                for w in range(WPT):
                    wd[:, t * WIN + b * WPT + w] = win_drel[t * WPT + w, b]
        if OVC:
            oi = np.zeros((128, NOVG * OG * OVC * 8 * 2), np.int16)
            od = np.full((128, NOVG * OG * OVC * 2), 255.0, np.float32)
            for og in range(NOVG):
                t0, t1 = og * OG, min((og + 1) * OG, cfg.TILES)
                ogg = t1 - t0
                base_i = og * OG * OVC * 8 * 2
                base_d = og * OG * OVC * 2
                for b in (0, 1):
                    chunks = []
                    for tl, t in enumerate(range(t0, t1)):
                        e_i, e_d = ov.get(
                            (t, b), (np.zeros(0, np.int16), np.zeros(0, np.float32))
                        )
                        cap = OVC * P
                        pi = np.zeros(cap, np.int16)
                        pd = np.full(cap, 255.0, np.float32)
                        pi[: len(e_i)] = e_i
                        pd[: len(e_d)] = e_d
                        chunks.append(pi)
                        for j in range(OVC):
                            od[:, base_d + b * ogg * OVC + tl * OVC + j] = pd[
                                j * P : (j + 1) * P
                            ]
                    stream = np.concatenate(chunks)
                    blk = _wrap16(stream)
                    off = base_i + b * ogg * OVC * 8
                    oi[:, off : off + blk.shape[1]] = blk
        else:
            oi = np.zeros((128, 16), np.int16)
            od = np.full((128, 2), 255.0, np.float32)
        per_shard.append(dict(win_idx=wi, win_drel=wd, ov_idx=oi, ov_drel=od))
    return per_shard, OVC, OG


# ---------------------------------------------------------------- bass kernel


def build_nc(cfg, OVC, OG, shapes):
    import concourse.bacc as bacc
    import concourse.bass as bass
    import concourse.tile as tile
    from concourse import mybir

    f32 = mybir.dt.float32
    i16 = mybir.dt.int16
    NOVG = math.ceil(cfg.TILES / OG)

    nc = bacc.Bacc("TRN2", target_bir_lowering=False, debug=False, num_devices=8)

    x_tab = nc.dram_tensor("x_tab", [cfg.N, FEAT], f32, kind="ExternalInput")
    wi_d = nc.dram_tensor("win_idx", list(shapes["win_idx"]), i16, kind="ExternalInput")
    wd_d = nc.dram_tensor("win_drel", list(shapes["win_drel"]), f32, kind="ExternalInput")
    oi_d = nc.dram_tensor("ov_idx", list(shapes["ov_idx"]), i16, kind="ExternalInput")
    od_d = nc.dram_tensor("ov_drel", list(shapes["ov_drel"]), f32, kind="ExternalInput")
    w1_d = nc.dram_tensor("W1", [FEAT, FEAT], f32, kind="ExternalInput")
    w2_d = nc.dram_tensor("W2", [FEAT, FEAT], f32, kind="ExternalInput")
    w3_d = nc.dram_tensor("W3", [FEAT, 1], f32, kind="ExternalInput")
    b1_d = nc.dram_tensor("b1", [FEAT], f32, kind="ExternalInput")
    b2_d = nc.dram_tensor("b2", [FEAT], f32, kind="ExternalInput")
    b3_d = nc.dram_tensor("b3", [1], f32, kind="ExternalInput")
    gam1_d = nc.dram_tensor("gamma1", [FEAT], f32, kind="ExternalInput")
    bet1_d = nc.dram_tensor("beta1", [FEAT], f32, kind="ExternalInput")
    gam2_d = nc.dram_tensor("gamma2", [FEAT], f32, kind="ExternalInput")
    bet2_d = nc.dram_tensor("beta2", [FEAT], f32, kind="ExternalInput")
    iota_w_d = nc.dram_tensor("iota_w", [P, WIN], f32, kind="ExternalInput")
    iota_p_d = nc.dram_tensor("iota_p", [P, P], f32, kind="ExternalInput")
    ident_d = nc.dram_tensor("ident", [P, P], f32, kind="ExternalInput")
    out_d = nc.dram_tensor("out", [cfg.SHARD], f32, kind="ExternalOutput")

    htab = [
        nc.dram_tensor(f"htab{i}", [cfg.N, FEAT], f32, kind="Internal")
        for i in range(2)
    ]
    shard_out = [
        nc.dram_tensor(f"shard_out{i}", [cfg.SHARD, FEAT], f32, kind="Internal")
        for i in range(2)
    ]
    stat_in = [
        nc.dram_tensor(f"stat_in{i}", [P, 2], f32, kind="Internal") for i in range(2)
    ]
    stat_out = [
        nc.dram_tensor(f"stat_out{i}", [P, 2], f32, kind="Internal") for i in range(2)
    ]

    AluOp = mybir.AluOpType
    ActF = mybir.ActivationFunctionType

    def bcast_inner(ap, inner):
        """ap [128, k] -> [128, k, inner] with 0-stride inner axis."""
        return bass.AP(
            tensor=ap.tensor,
            offset=ap.offset,
            ap=[list(ap.ap[0]), list(ap.ap[1]), [0, inner]],
        )

    def bcast_rep(ap, reps):
        """ap [128, k] -> [128, reps, k] with 0-stride middle axis."""
        return bass.AP(
            tensor=ap.tensor,
            offset=ap.offset,
            ap=[list(ap.ap[0]), [0, reps], list(ap.ap[1])],
        )

    with tile.TileContext(nc) as tc:
        with (
            tc.tile_pool(name="consts", bufs=1) as consts,
            tc.tile_pool(name="gwin", bufs=4) as gwinp,
            tc.tile_pool(name="gov", bufs=2) as govp,
            tc.tile_pool(name="ohp", bufs=4) as ohp,
            tc.tile_pool(name="aggp", bufs=3) as aggp,
            tc.tile_pool(name="hraw", bufs=1) as hrawp,
            tc.tile_pool(name="statp", bufs=2) as statp,
            tc.tile_pool(name="small", bufs=8) as small,
            tc.tile_pool(name="p2", bufs=3) as p2p,
            tc.tile_pool(name="outp", bufs=1) as outp,
            tc.tile_pool(name="ps_agg", bufs=2, space="PSUM") as ps_agg,
            tc.tile_pool(name="ps_h", bufs=2, space="PSUM") as ps_h,
            tc.tile_pool(name="ps_t", bufs=2, space="PSUM") as ps_t,
        ):
            # ---- layer-invariant inputs (indices, dest_rel, weights, consts)
            wi_sb = consts.tile(list(shapes["win_idx"]), i16, tag="wi")
            nc.sync.dma_start(out=wi_sb[:], in_=wi_d[:])
            wd_sb = consts.tile(list(shapes["win_drel"]), f32, tag="wd")
            nc.sync.dma_start(out=wd_sb[:], in_=wd_d[:])
            oi_sb = consts.tile(list(shapes["ov_idx"]), i16, tag="oi")
            nc.sync.dma_start(out=oi_sb[:], in_=oi_d[:])
            od_sb = consts.tile(list(shapes["ov_drel"]), f32, tag="od")
            nc.sync.dma_start(out=od_sb[:], in_=od_d[:])

            w_sb = []
            for wdr in (w1_d, w2_d):
                t = consts.tile([P, FEAT], f32, tag=f"w_{wdr.name}")
                nc.sync.dma_start(out=t[:], in_=wdr[:])
                w_sb.append(t)
            w3_sb = consts.tile([P, 1], f32, tag="w3")
            nc.sync.dma_start(out=w3_sb[:], in_=w3_d[:])
            b_sb = []
            for bd in (b1_d, b2_d):
                t = consts.tile([P, 1], f32, tag=f"b_{bd.name}")
                nc.sync.dma_start(out=t[:], in_=bd[:, None])
                b_sb.append(t)
            b3_sb = consts.tile([P, 1], f32, tag="b3")
            nc.sync.dma_start(out=b3_sb[:], in_=b3_d[:].to_broadcast([P, 1]))
            gb_sb = []
            for gd, bd in ((gam1_d, bet1_d), (gam2_d, bet2_d)):
                tg_ = consts.tile([P, 1], f32, tag=f"g_{gd.name}")
                nc.sync.dma_start(out=tg_[:], in_=gd[:, None])
                tb_ = consts.tile([P, 1], f32, tag=f"be_{bd.name}")
                nc.sync.dma_start(out=tb_[:], in_=bd[:, None])
                gb_sb.append((tg_, tb_))
            iota_w = consts.tile([P, WIN], f32, tag="iota_w")
            nc.sync.dma_start(out=iota_w[:], in_=iota_w_d[:])
            iota_p = consts.tile([P, P], f32, tag="iota_p")
            nc.sync.dma_start(out=iota_p[:], in_=iota_p_d[:])
            ident = consts.tile([P, P], f32, tag="ident")
            nc.sync.dma_start(out=ident[:], in_=ident_d[:])
            eps_sb = consts.tile([P, 1], f32, tag="eps")
            nc.vector.memset(eps_sb[:], cfg.EPS)

            for layer in range(cfg.LAYERS):
                table = x_tab if layer == 0 else htab[layer - 1]
                is_last = layer == cfg.LAYERS - 1
                if not is_last:
                    hraw = hrawp.tile([P, cfg.TILES * P], f32, tag="hraw")
                    stat_t = statp.tile([P, cfg.TILES, 6], f32, tag="stats")
                else:
                    out_sb = outp.tile([P, cfg.TILES], f32, tag="outsb")

                for og in range(NOVG):
                    t0, t1 = og * OG, min((og + 1) * OG, cfg.TILES)
                    ogg = t1 - t0
                    gov = []
                    if OVC:
                        nch = ogg * OVC
                        base_i = og * OG * OVC * 8 * 2
                        for b in (0, 1):
                            gt = govp.tile([P, OG * OVC, FEAT], f32, tag=f"gov{b}")
                            src = table[:, :] if b == 0 else table[cfg.HALF :, :]
                            nc.gpsimd.dma_gather(
                                gt[:, :nch, :],
                                src,
                                oi_sb[:, base_i + b * nch * 8 : base_i + (b + 1) * nch * 8],
                                nch * P,
                                nch * P,
                                FEAT,
                            )
                            gov.append(gt)

                    for t in range(t0, t1):
                        tl = t - t0
                        gwin = []
                        for b in (0, 1):
                            gt = gwinp.tile([P, WPT, FEAT], f32, tag=f"gwin{b}")
                            src = table[:, :] if b == 0 else table[cfg.HALF :, :]
                            nc.gpsimd.dma_gather(
                                gt[:],
                                src,
                                wi_sb[:, t * P + b * 64 : t * P + (b + 1) * 64],
                                WPT * P,
                                WPT * P,
                                FEAT,
                            )
                            gwin.append(gt)
                        oh_w = []
                        oh_o = []
                        for b in (0, 1):
                            t_ohw = ohp.tile([P, WPT * WIN], f32, tag=f"ohw{b}")
                            nc.vector.tensor_tensor(
                                out=t_ohw[:],
                                in0=bcast_inner(
                                    wd_sb[:, t * WIN + b * WPT : t * WIN + (b + 1) * WPT],
                                    WIN,
                                ),
                                in1=bcast_rep(iota_w[:], WPT),
                                op=AluOp.is_equal,
                            )
                            oh_w.append(t_ohw)
                            if OVC:
                                base_d = og * OG * OVC * 2
                                c0 = base_d + b * ogg * OVC + tl * OVC
                                t_oho = ohp.tile([P, OVC * P], f32, tag=f"oho{b}")
                                nc.vector.tensor_tensor(
                                    out=t_oho[:],
                                    in0=bcast_inner(od_sb[:, c0 : c0 + OVC], P),
                                    in1=bcast_rep(iota_p[:], OVC),
                                    op=AluOp.is_equal,
                                )
                                oh_o.append(t_oho)

                        agg_ps = ps_agg.tile([P, P], f32, tag="agg")
                        n_ov = 2 * OVC
                        for b in (0, 1):
                            for w in range(WPT):
                                nc.tensor.matmul(
                                    agg_ps[:, w * WIN : (w + 1) * WIN],
                                    lhsT=gwin[b][:, w, :],
                                    rhs=oh_w[b][:, w * WIN : (w + 1) * WIN],
                                    start=(b == 0 and w == 0),
                                    stop=(n_ov == 0 and b == 1 and w == WPT - 1),
                                )
                        k_ov = 0
                        for b in range(2):
                            for j in range(OVC):
                                k_ov += 1
                                nc.tensor.matmul(
                                    agg_ps[:, :],
                                    lhsT=gov[b][:, tl * OVC + j, :],
                                    rhs=oh_o[b][:, j * P : (j + 1) * P],
                                    start=False,
                                    stop=(k_ov == n_ov),
                                )

                        agg_sb = aggp.tile([P, P], f32, tag="aggsb")
                        nc.vector.tensor_copy(out=agg_sb[:], in_=agg_ps[:])
                        valid = cfg.VALID_LAST if t == cfg.TILES - 1 else P
                        if not is_last:
                            h_ps = ps_h.tile([P, P], f32, tag="hps")
                            nc.tensor.matmul(
                                h_ps[:], lhsT=w_sb[layer][:], rhs=agg_sb[:],
                                start=True, stop=True,
                            )
                            nc.vector.tensor_scalar_add(
                                out=hraw[:, t * P : t * P + P],
                                in0=h_ps[:],
                                scalar1=b_sb[layer][:],
                            )
                            nc.vector.bn_stats(
                                out=stat_t[:, t, :], in_=hraw[:, t * P : t * P + valid]
                            )
                        else:
                            o_ps = ps_h.tile([P, 1], f32, tag="hps")
                            nc.tensor.matmul(
                                o_ps[:], lhsT=agg_sb[:], rhs=w3_sb[:],
                                start=True, stop=True,
                            )
                            nc.vector.tensor_scalar_add(
                                out=out_sb[:, t : t + 1], in0=o_ps[:], scalar1=b3_sb[:]
                            )

                if not is_last:
                    # ---- global BN stats
                    mv = small.tile([P, 2], f32, tag="mv")
                    nc.vector.bn_aggr(out=mv[:], in_=stat_t[:, :, :])
                    sloc = small.tile([P, 2], f32, tag="sloc")
                    nc.vector.tensor_copy(out=sloc[:, 0:1], in_=mv[:, 0:1])
                    nc.vector.tensor_tensor(
                        out=sloc[:, 1:2], in0=mv[:, 0:1], in1=mv[:, 0:1], op=AluOp.mult
                    )
                    nc.vector.tensor_add(
                        out=sloc[:, 1:2], in0=sloc[:, 1:2], in1=mv[:, 1:2]
                    )
                    nc.sync.dma_start(out=stat_in[layer][:], in_=sloc[:])
                    if cfg.USE_AR:
                        nc.gpsimd.collective_compute(
                            "AllReduce",
                            AluOp.add,
                            replica_groups=[[0, 1, 2, 3, 4, 5, 6, 7]],
                            ins=[stat_in[layer][:]],
                            outs=[stat_out[layer][:]],
                        )
                    else:
                        nc.sync.dma_start(out=stat_out[layer][:], in_=stat_in[layer][:])
                    sglob = small.tile([P, 2], f32, tag="sglob")
                    nc.sync.dma_start(out=sglob[:], in_=stat_out[layer][:])
                    nc.scalar.mul(
                        out=sglob[:], in_=sglob[:], mul=0.125 if cfg.USE_AR else 1.0
                    )
                    var = small.tile([P, 1], f32, tag="var")
                    nc.vector.tensor_tensor(
                        out=var[:], in0=sglob[:, 0:1], in1=sglob[:, 0:1], op=AluOp.mult
                    )
                    nc.vector.tensor_sub(out=var[:], in0=sglob[:, 1:2], in1=var[:])
                    rstd = small.tile([P, 1], f32, tag="rstd")
                    nc.scalar.activation(
                        out=rstd[:], in_=var[:], func=ActF.Sqrt, bias=eps_sb[:]
                    )
                    nc.vector.reciprocal(out=rstd[:], in_=rstd[:])
                    scal = small.tile([P, 1], f32, tag="scal")
                    nc.vector.tensor_tensor(
                        out=scal[:], in0=gb_sb[layer][0][:], in1=rstd[:], op=AluOp.mult
                    )
                    shif = small.tile([P, 1], f32, tag="shif")
                    nc.vector.tensor_tensor(
                        out=shif[:], in0=sglob[:, 0:1], in1=scal[:], op=AluOp.mult
                    )
                    nc.vector.tensor_sub(out=shif[:], in0=gb_sb[layer][1][:], in1=shif[:])
                    # ---- pass 2: BN + relu + transpose + write shard rows
                    for t in range(cfg.TILES):
                        hbn = p2p.tile([P, P], f32, tag="hbn")
                        nc.scalar.activation(
                            out=hbn[:],
                            in_=hraw[:, t * P : (t + 1) * P],
                            func=ActF.Relu,
                            bias=shif[:],
                            scale=scal[:],
                        )
                        t_ps = ps_t.tile([P, P], f32, tag="tps")
                        nc.tensor.transpose(out=t_ps[:], in_=hbn[:], identity=ident[:])
                        hrow = p2p.tile([P, P], f32, tag="hrow")
                        nc.vector.tensor_copy(out=hrow[:], in_=t_ps[:])
                        valid = cfg.VALID_LAST if t == cfg.TILES - 1 else P
                        nc.sync.dma_start(
                            out=shard_out[layer][t * P : t * P + valid, :],
                            in_=hrow[:valid, :],
                        )
                    if cfg.USE_AG:
                        nc.gpsimd.collective_compute(
                            "AllGather",
                            AluOp.bypass,
                            replica_groups=[[0, 1, 2, 3], [4, 5, 6, 7]],
                            ins=[shard_out[layer][:]],
                            outs=[htab[layer][:]],
                        )
                    else:
                        nc.sync.dma_start(
                            out=htab[layer][0 : cfg.SHARD, :], in_=shard_out[layer][:]
                        )
                else:
                    nfull = cfg.TILES - 1
                    if nfull:
                        nc.sync.dma_start(
                            out=out_d[0 : nfull * P].rearrange("(t p) -> p t", p=P),
                            in_=out_sb[:, 0:nfull],
                        )
                    nc.sync.dma_start(
                        out=out_d[nfull * P : cfg.SHARD, None],
                        in_=out_sb[: cfg.VALID_LAST, nfull : nfull + 1],
                    )

    nc.compile()
    return nc


# ---------------------------------------------------------------- consts + run


def _const_inputs():
    iota_w = np.tile(np.arange(WIN, dtype=np.float32), (P, 1))
    iota_p = np.tile(np.arange(P, dtype=np.float32), (P, 1))
    ident = np.eye(P, dtype=np.float32)
    return iota_w, iota_p, ident


def run_gcn(cfg, inputs, trace=False):
    from concourse.bass_utils import run_bass_kernel_spmd

    x = np.asarray(inputs["x"], dtype=np.float32)
    edge_index = np.asarray(inputs["edge_index"])
    per_shard, OVC, OG = build_schedule(cfg, edge_index)
    shapes = {k: v.shape for k, v in per_shard[0].items()}
    nc = build_nc(cfg, OVC, OG, shapes)

    iota_w, iota_p, ident = _const_inputs()
    common = {
        "W1": np.asarray(inputs["W1"], np.float32),
        "W2": np.asarray(inputs["W2"], np.float32),
        "W3": np.asarray(inputs["W3"], np.float32),
        "b1": np.asarray(inputs["b1"], np.float32),
        "b2": np.asarray(inputs["b2"], np.float32),
        "b3": np.asarray(inputs["b3"], np.float32),
        "gamma1": np.asarray(inputs["gamma1"], np.float32),
        "beta1": np.asarray(inputs["beta1"], np.float32),
        "gamma2": np.asarray(inputs["gamma2"], np.float32),
        "beta2": np.asarray(inputs["beta2"], np.float32),
        "iota_w": iota_w,
        "iota_p": iota_p,
        "ident": ident,
    }
    in_maps = []
    for c in range(8):
        b, q = c // 4, c % 4
        m = dict(common)
        m["x_tab"] = np.ascontiguousarray(x[b])
        m.update(per_shard[q])
        in_maps.append(m)

    try:
        res = run_bass_kernel_spmd(nc, in_maps, core_ids=list(range(8)), trace=trace)
    except ModuleNotFoundError:
        res = run_bass_kernel_spmd(nc, in_maps, core_ids=list(range(8)), trace=False)
    out = np.empty((cfg.BATCH, 4 * cfg.SHARD), np.float32)
    for c in range(8):
        b, q = c // 4, c % 4
        out[b, q * cfg.SHARD : (q + 1) * cfg.SHARD] = res.results[c]["out"]
    return out, res


def kernel(**inputs) -> np.ndarray:
    cfg = Cfg()
    out, _ = run_gcn(cfg, inputs, trace=False)
    return out



# revision 7
# speedup vs baseline: 1.8869x; 1.8869x over previous
"""ClusterGCN (3-layer GCN, sum-aggregation) on 8 Trainium2 NeuronCores.

Strategy (hardcoded for B=2, N=50000, F=H=128, E=800000, 8 cores):
  - Batch-merged tables: node row = [h_b0(128) | h_b1(128)] bf16 (512 B), so
    ONE dma_gather descriptor per edge serves both batch elements. SWDGE
    descriptor generation on GpSimd (~8 ns/row) is the bottleneck; halving
    descriptors nearly halves the kernel.
  - core c owns destination nodes [c*6250, (c+1)*6250) for BOTH batches.
  - Reassociate each layer: A @ (h @ W) == (A @ h) @ W, so every layer is
    gather-aggregate (segment-sum over edges) followed by a dense 128x128
    matmul per batch. Aggregation output lives as agg_T[f, d] bf16 matmuls
    accumulating into two PSUM tiles (one per batch).
  - Per-edge gathers use the SWDGE dma_gather custom instruction (int16
    indices, max 1024 per call). Indices only reach 32767 rows, so each edge
    stream is split into a low (src < 25000) and high (src >= 25000) bucket
    gathered from offset views of the table.
  - Segment-sum on the tensor engine: edges are grouped per 16-destination
    window into 128-slot chunks; chunk x onehot(dest_rel) matmuls accumulate
    into PSUM tiles of 128 destinations. Window overflow edges go to
    per-tile overflow chunks gathered in batched calls across tile groups.
  - BatchNorm is training-mode over all B*N rows: per-core bn_stats/bn_aggr,
    then an 8-core AllReduce of (mean, E[x^2]).
  - After BN+ReLU the shard rows are transposed back to row-major bf16 and
    AllGathered across all 8 cores into the next gather table.
"""

import math

import numpy as np

# ---------------------------------------------------------------- config

P = 128
FEAT = 128  # per-batch feature width
TW = 256  # merged table row width (2 batches)
WIN = 16
WPT = P // WIN  # windows per tile (8); one window call = WPT*128 = 1024 idx
NSHARD = 8


class Cfg:
    def __init__(self, n_nodes=50000, batch=2, eps=1e-5):
        assert n_nodes % NSHARD == 0
        self.N = n_nodes
        self.SHARD = n_nodes // NSHARD
        self.BATCH = batch
        self.HALF = n_nodes // 2
        assert self.HALF <= 32767
        self.TILES = math.ceil(self.SHARD / P)
        self.VALID_LAST = self.SHARD - (self.TILES - 1) * P
        self.EPS = eps
        # debug toggles
        self.LAYERS = 3
        self.USE_AR = True
        self.USE_AG = True


# ---------------------------------------------------------------- host schedule


def _shard_schedule(cfg, row, col, q):
    """Per-shard edge schedule: window slots + overflow lists.

    Returns (win_idx [NW,2,128] int16, win_drel [NW,2,128] f32,
             ov: dict[(tile, bucket)] -> (idx16 1d, drel 1d))."""
    base = q * cfg.SHARD
    m = (col >= base) & (col < base + cfg.SHARD)
    r = row[m].astype(np.int64)
    c = (col[m] - base).astype(np.int64)
    wg = c // WIN
    bkt = (r >= cfg.HALF).astype(np.int64)
    key = wg * 2 + bkt
    order = np.argsort(key, kind="stable")
    r, c, wg, bkt, key = r[order], c[order], wg[order], bkt[order], key[order]
    n = len(key)
    NW = cfg.TILES * WPT

    if n == 0:
        return (
            np.zeros((NW, 2, P), np.int16),
            np.full((NW, 2, P), 255.0, np.float32),
            {},
        )

    newg = np.empty(n, bool)
    newg[0] = True
    newg[1:] = key[1:] != key[:-1]
    gstart = np.flatnonzero(newg)
    counts = np.diff(np.append(gstart, n))
    starts = np.repeat(gstart, counts)
    pos = np.arange(n) - starts
    idx16 = np.where(bkt == 1, r - cfg.HALF, r).astype(np.int16)

    inw = pos < P
    win_idx = np.zeros((NW, 2, P), np.int16)
    win_drel = np.full((NW, 2, P), 255.0, np.float32)
    win_idx[wg[inw], bkt[inw], pos[inw]] = idx16[inw]
    win_drel[wg[inw], bkt[inw], pos[inw]] = (c[inw] - wg[inw] * WIN).astype(np.float32)

    ov = {}
    ow = ~inw
    if ow.any():
        t_ov = wg[ow] // WPT
        b_ov = bkt[ow]
        i_ov = idx16[ow]
        d_ov = (c[ow] - t_ov * P).astype(np.float32)
        okey = t_ov * 2 + b_ov
        oorder = np.argsort(okey, kind="stable")
        t_ov, b_ov, i_ov, d_ov, okey = (
            t_ov[oorder],
            b_ov[oorder],
            i_ov[oorder],
            d_ov[oorder],
            okey[oorder],
        )
        bounds = np.flatnonzero(np.append(True, okey[1:] != okey[:-1]))
        bounds = np.append(bounds, len(okey))
        for j in range(len(bounds) - 1):
            s, e = bounds[j], bounds[j + 1]
            ov[(int(t_ov[s]), int(b_ov[s]))] = (i_ov[s:e], d_ov[s:e])
    return win_idx, win_drel, ov


def _wrap16(stream):
    """[n] idx stream -> [128, n/16] wrapped col-major, replicated x8."""
    return np.tile(stream.reshape(-1, 16).T, (8, 1))


def build_schedule(cfg, edge_index):
    """Build gather-index / dest-rel input tensors for the 8 shards.

    Layout (layer-invariant, loaded once):
      win_idx  [128, TILES*128] i16 : tile t -> cols [t*128, t*128+64) = lo
               window stream (8 chunks), [+64, +128) = hi stream.
      win_drel [128, TILES*16] f32  : tile t -> cols [t*16+w] lo, [t*16+8+w] hi.
      ov_idx   [128, NOVG*OG*OVC*8*2] i16 : group og -> lo block then hi block.
      ov_drel  [128, NOVG*OG*OVC*2] f32   : group og -> lo cols then hi cols.

    Returns (per_shard list of dicts, OVC, OG)."""
    row = np.asarray(edge_index[0])
    col = np.asarray(edge_index[1])
    shards = [_shard_schedule(cfg, row, col, q) for q in range(NSHARD)]

    ovc = 0
    for _, _, ov in shards:
        for (t, b), (i1, _) in ov.items():
            ovc = max(ovc, math.ceil(len(i1) / P))
    OVC = ovc
    OG = max(1, WPT // OVC) if OVC else 1
    assert OG * OVC <= WPT, f"overflow call too large: OG={OG} OVC={OVC}"
    NOVG = math.ceil(cfg.TILES / OG)

    per_shard = []
    for win_idx, win_drel, ov in shards:
        wi = np.zeros((128, cfg.TILES * P), np.int16)
        wd = np.full((128, cfg.TILES * WIN), 255.0, np.float32)
        for t in range(cfg.TILES):
            for b in (0, 1):
                stream = np.concatenate(
                    [win_idx[t * WPT + w, b] for w in range(WPT)]
                )
                wi[:, t * P + b * 64 : t * P + (b + 1) * 64] = _wrap16(stream)
                for w in range(WPT):
                    wd[:, t * WIN + b * WPT + w] = win_drel[t * WPT + w, b]
        if OVC:
            oi = np.zeros((128, NOVG * OG * OVC * 8 * 2), np.int16)
            od = np.full((128, NOVG * OG * OVC * 2), 255.0, np.float32)
            for og in range(NOVG):
                t0, t1 = og * OG, min((og + 1) * OG, cfg.TILES)
                ogg = t1 - t0
                base_i = og * OG * OVC * 8 * 2
                base_d = og * OG * OVC * 2
                for b in (0, 1):
                    chunks = []
                    for tl, t in enumerate(range(t0, t1)):
                        e_i, e_d = ov.get(
                            (t, b), (np.zeros(0, np.int16), np.zeros(0, np.float32))
                        )
                        cap = OVC * P
                        pi = np.zeros(cap, np.int16)
                        pd = np.full(cap, 255.0, np.float32)
                        pi[: len(e_i)] = e_i
                        pd[: len(e_d)] = e_d
                        chunks.append(pi)
                        for j in range(OVC):
                            od[:, base_d + b * ogg * OVC + tl * OVC + j] = pd[
                                j * P : (j + 1) * P
                            ]
                    stream = np.concatenate(chunks)
                    blk = _wrap16(stream)
                    off = base_i + b * ogg * OVC * 8
                    oi[:, off : off + blk.shape[1]] = blk
        else:
            oi = np.zeros((128, 16), np.int16)
            od = np.full((128, 2), 255.0, np.float32)
        per_shard.append(dict(win_idx=wi, win_drel=wd, ov_idx=oi, ov_drel=od))
    return per_shard, OVC, OG


# ---------------------------------------------------------------- bass kernel


def build_nc(cfg, OVC, OG, shapes):
    import concourse.bacc as bacc
    import concourse.bass as bass
    import concourse.tile as tile
    from concourse import mybir

    f32 = mybir.dt.float32
    bf16 = mybir.dt.bfloat16
    i16 = mybir.dt.int16
    NOVG = math.ceil(cfg.TILES / OG)

    nc = bacc.Bacc("TRN2", target_bir_lowering=False, debug=False, num_devices=8)

    # merged bf16 node table for layer 0 (built on host from x)
    xm_d = nc.dram_tensor("xm_tab", [cfg.N, TW], bf16, kind="ExternalInput")
    wi_d = nc.dram_tensor("win_idx", list(shapes["win_idx"]), i16, kind="ExternalInput")
    wd_d = nc.dram_tensor("win_drel", list(shapes["win_drel"]), f32, kind="ExternalInput")
    oi_d = nc.dram_tensor("ov_idx", list(shapes["ov_idx"]), i16, kind="ExternalInput")
    od_d = nc.dram_tensor("ov_drel", list(shapes["ov_drel"]), f32, kind="ExternalInput")
    w1_d = nc.dram_tensor("W1", [FEAT, FEAT], bf16, kind="ExternalInput")
    w2_d = nc.dram_tensor("W2", [FEAT, FEAT], bf16, kind="ExternalInput")
    w3_d = nc.dram_tensor("W3", [FEAT, 1], bf16, kind="ExternalInput")
    b1_d = nc.dram_tensor("b1", [FEAT], f32, kind="ExternalInput")
    b2_d = nc.dram_tensor("b2", [FEAT], f32, kind="ExternalInput")
    b3_d = nc.dram_tensor("b3", [1], f32, kind="ExternalInput")
    gam1_d = nc.dram_tensor("gamma1", [FEAT], f32, kind="ExternalInput")
    bet1_d = nc.dram_tensor("beta1", [FEAT], f32, kind="ExternalInput")
    gam2_d = nc.dram_tensor("gamma2", [FEAT], f32, kind="ExternalInput")
    bet2_d = nc.dram_tensor("beta2", [FEAT], f32, kind="ExternalInput")
    iota_w_d = nc.dram_tensor("iota_w", [P, WIN], f32, kind="ExternalInput")
    iota_p_d = nc.dram_tensor("iota_p", [P, P], f32, kind="ExternalInput")
    ident_d = nc.dram_tensor("ident", [P, P], bf16, kind="ExternalInput")
    out0_d = nc.dram_tensor("out0", [cfg.SHARD], f32, kind="ExternalOutput")
    out1_d = nc.dram_tensor("out1", [cfg.SHARD], f32, kind="ExternalOutput")

    htab = [
        nc.dram_tensor(f"htab{i}", [cfg.N, TW], bf16, kind="Internal")
        for i in range(2)
    ]
    shard_out = [
        nc.dram_tensor(f"shard_out{i}", [cfg.SHARD, TW], bf16, kind="Internal")
        for i in range(2)
    ]
    stat_in = [
        nc.dram_tensor(f"stat_in{i}", [P, 2], f32, kind="Internal") for i in range(2)
    ]
    stat_out = [
        nc.dram_tensor(f"stat_out{i}", [P, 2], f32, kind="Internal") for i in range(2)
    ]

    AluOp = mybir.AluOpType
    ActF = mybir.ActivationFunctionType

    def bcast_inner(ap, inner):
        """ap [128, k] -> [128, k, inner] with 0-stride inner axis."""
        return bass.AP(
            tensor=ap.tensor,
            offset=ap.offset,
            ap=[list(ap.ap[0]), list(ap.ap[1]), [0, inner]],
        )

    def bcast_rep(ap, reps):
        """ap [128, k] -> [128, reps, k] with 0-stride middle axis."""
        return bass.AP(
            tensor=ap.tensor,
            offset=ap.offset,
            ap=[list(ap.ap[0]), [0, reps], list(ap.ap[1])],
        )

    with tile.TileContext(nc) as tc:
        with (
            tc.tile_pool(name="consts", bufs=1) as consts,
            tc.tile_pool(name="gwin", bufs=6) as gwinp,
            tc.tile_pool(name="gov", bufs=3) as govp,
            tc.tile_pool(name="ohp", bufs=4) as ohp,
            tc.tile_pool(name="aggp", bufs=4) as aggp,
            tc.tile_pool(name="hraw", bufs=1) as hrawp,
            tc.tile_pool(name="statp", bufs=2) as statp,
            tc.tile_pool(name="small", bufs=8) as small,
            tc.tile_pool(name="p2", bufs=3) as p2p,
            tc.tile_pool(name="outp", bufs=1) as outp,
            tc.tile_pool(name="ps_agg", bufs=2, space="PSUM") as ps_agg,
            tc.tile_pool(name="ps_h", bufs=2, space="PSUM") as ps_h,
            tc.tile_pool(name="ps_t", bufs=2, space="PSUM") as ps_t,
        ):
            # ---- layer-invariant inputs (indices, dest_rel, weights, consts)
            wi_sb = consts.tile(list(shapes["win_idx"]), i16, tag="wi")
            nc.sync.dma_start(out=wi_sb[:], in_=wi_d[:])
            wd_sb = consts.tile(list(shapes["win_drel"]), f32, tag="wd")
            nc.sync.dma_start(out=wd_sb[:], in_=wd_d[:])
            oi_sb = consts.tile(list(shapes["ov_idx"]), i16, tag="oi")
            nc.sync.dma_start(out=oi_sb[:], in_=oi_d[:])
            od_sb = consts.tile(list(shapes["ov_drel"]), f32, tag="od")
            nc.sync.dma_start(out=od_sb[:], in_=od_d[:])

            w_sb = []
            for wdr in (w1_d, w2_d):
                t = consts.tile([P, FEAT], bf16, tag=f"w_{wdr.name}")
                nc.sync.dma_start(out=t[:], in_=wdr[:])
                w_sb.append(t)
            w3_sb = consts.tile([P, 1], bf16, tag="w3")
            nc.sync.dma_start(out=w3_sb[:], in_=w3_d[:])
            b_sb = []
            for bd in (b1_d, b2_d):
                t = consts.tile([P, 1], f32, tag=f"b_{bd.name}")
                nc.sync.dma_start(out=t[:], in_=bd[:, None])
                b_sb.append(t)
            b3_sb = consts.tile([P, 1], f32, tag="b3")
            nc.sync.dma_start(out=b3_sb[:], in_=b3_d[:].to_broadcast([P, 1]))
            gb_sb = []
            for gd, bd in ((gam1_d, bet1_d), (gam2_d, bet2_d)):
                tg_ = consts.tile([P, 1], f32, tag=f"g_{gd.name}")
                nc.sync.dma_start(out=tg_[:], in_=gd[:, None])
                tb_ = consts.tile([P, 1], f32, tag=f"be_{bd.name}")
                nc.sync.dma_start(out=tb_[:], in_=bd[:, None])
                gb_sb.append((tg_, tb_))
            iota_w = consts.tile([P, WIN], f32, tag="iota_w")
            nc.sync.dma_start(out=iota_w[:], in_=iota_w_d[:])
            iota_p = consts.tile([P, P], f32, tag="iota_p")
            nc.sync.dma_start(out=iota_p[:], in_=iota_p_d[:])
            ident = consts.tile([P, P], bf16, tag="ident")
            nc.sync.dma_start(out=ident[:], in_=ident_d[:])
            eps_sb = consts.tile([P, 1], f32, tag="eps")
            nc.vector.memset(eps_sb[:], cfg.EPS)

            for layer in range(cfg.LAYERS):
                table = xm_d if layer == 0 else htab[layer - 1]
                is_last = layer == cfg.LAYERS - 1
                if not is_last:
                    # hraw: [fout, dest] per batch, f32
                    hraw = [
                        hrawp.tile(
                            [P, cfg.TILES * P], f32, tag=f"hraw{bat}",
                            name=f"hraw{bat}",
                        )
                        for bat in range(2)
                    ]
                    stat_t = statp.tile([P, 2 * cfg.TILES, 6], f32, tag="stats")
                else:
                    out_sb = [
                        outp.tile(
                            [P, cfg.TILES], f32, tag=f"outsb{bat}",
                            name=f"outsb{bat}",
                        )
                        for bat in range(2)
                    ]

                for og in range(NOVG):
                    t0, t1 = og * OG, min((og + 1) * OG, cfg.TILES)
                    ogg = t1 - t0
                    gov = []
                    if OVC:
                        nch = ogg * OVC
                        base_i = og * OG * OVC * 8 * 2
                        for b in (0, 1):
                            gt = govp.tile([P, OG * OVC, TW], bf16, tag=f"gov{b}")
                            src = table[:, :] if b == 0 else table[cfg.HALF :, :]
                            nc.gpsimd.dma_gather(
                                gt[:, :nch, :],
                                src,
                                oi_sb[:, base_i + b * nch * 8 : base_i + (b + 1) * nch * 8],
                                nch * P,
                                nch * P,
                                TW,
                            )
                            gov.append(gt)

                    for t in range(t0, t1):
                        tl = t - t0
                        gwin = []
                        for b in (0, 1):
                            gt = gwinp.tile([P, WPT, TW], bf16, tag=f"gwin{b}")
                            src = table[:, :] if b == 0 else table[cfg.HALF :, :]
                            nc.gpsimd.dma_gather(
                                gt[:],
                                src,
                                wi_sb[:, t * P + b * 64 : t * P + (b + 1) * 64],
                                WPT * P,
                                WPT * P,
                                TW,
                            )
                            gwin.append(gt)
                        oh_w = []
                        oh_o = []
                        for b in (0, 1):
                            t_ohw = ohp.tile([P, WPT * WIN], bf16, tag=f"ohw{b}")
                            nc.vector.tensor_tensor(
                                out=t_ohw[:],
                                in0=bcast_inner(
                                    wd_sb[:, t * WIN + b * WPT : t * WIN + (b + 1) * WPT],
                                    WIN,
                                ),
                                in1=bcast_rep(iota_w[:], WPT),
                                op=AluOp.is_equal,
                            )
                            oh_w.append(t_ohw)
                            if OVC:
                                base_d = og * OG * OVC * 2
                                c0 = base_d + b * ogg * OVC + tl * OVC
                                t_oho = ohp.tile([P, OVC * P], bf16, tag=f"oho{b}")
                                nc.vector.tensor_tensor(
                                    out=t_oho[:],
                                    in0=bcast_inner(od_sb[:, c0 : c0 + OVC], P),
                                    in1=bcast_rep(iota_p[:], OVC),
                                    op=AluOp.is_equal,
                                )
                                oh_o.append(t_oho)

                        # two PSUM accumulators, one per batch element
                        agg_ps = [
                            ps_agg.tile(
                                [P, P], f32, tag=f"agg{bat}", name=f"agg{bat}"
                            )
                            for bat in range(2)
                        ]
                        n_ov = 2 * OVC
                        for bat in range(2):
                            fsl = slice(bat * FEAT, (bat + 1) * FEAT)
                            for b in (0, 1):
                                for w in range(WPT):
                                    nc.tensor.matmul(
                                        agg_ps[bat][:, w * WIN : (w + 1) * WIN],
                                        lhsT=gwin[b][:, w, fsl],
                                        rhs=oh_w[b][:, w * WIN : (w + 1) * WIN],
                                        start=(b == 0 and w == 0),
                                        stop=(n_ov == 0 and b == 1 and w == WPT - 1),
                                    )
                            k_ov = 0
                            for b in range(2):
                                for j in range(OVC):
                                    k_ov += 1
                                    nc.tensor.matmul(
                                        agg_ps[bat][:, :],
                                        lhsT=gov[b][:, tl * OVC + j, fsl],
                                        rhs=oh_o[b][:, j * P : (j + 1) * P],
                                        start=False,
                                        stop=(k_ov == n_ov),
                                    )

                        valid = cfg.VALID_LAST if t == cfg.TILES - 1 else P
                        for bat in range(2):
                            agg_sb = aggp.tile([P, P], bf16, tag=f"aggsb{bat}")
                            nc.vector.tensor_copy(out=agg_sb[:], in_=agg_ps[bat][:])
                            if not is_last:
                                h_ps = ps_h.tile([P, P], f32, tag="hps")
                                nc.tensor.matmul(
                                    h_ps[:], lhsT=w_sb[layer][:], rhs=agg_sb[:],
                                    start=True, stop=True,
                                )
                                nc.vector.tensor_scalar_add(
                                    out=hraw[bat][:, t * P : t * P + P],
                                    in0=h_ps[:],
                                    scalar1=b_sb[layer][:],
                                )
                                nc.vector.bn_stats(
                                    out=stat_t[:, 2 * t + bat, :],
                                    in_=hraw[bat][:, t * P : t * P + valid],
                                )
                            else:
                                o_ps = ps_h.tile([P, 1], f32, tag="hps")
                                nc.tensor.matmul(
                                    o_ps[:], lhsT=agg_sb[:], rhs=w3_sb[:],
                                    start=True, stop=True,
                                )
                                nc.vector.tensor_scalar_add(
                                    out=out_sb[bat][:, t : t + 1],
                                    in0=o_ps[:],
                                    scalar1=b3_sb[:],
                                )

                if not is_last:
                    # ---- global BN stats
                    mv = small.tile([P, 2], f32, tag="mv")
                    nc.vector.bn_aggr(out=mv[:], in_=stat_t[:, :, :])
                    sloc = small.tile([P, 2], f32, tag="sloc")
                    nc.vector.tensor_copy(out=sloc[:, 0:1], in_=mv[:, 0:1])
                    nc.vector.tensor_tensor(
                        out=sloc[:, 1:2], in0=mv[:, 0:1], in1=mv[:, 0:1], op=AluOp.mult
                    )
                    nc.vector.tensor_add(
                        out=sloc[:, 1:2], in0=sloc[:, 1:2], in1=mv[:, 1:2]
                    )
                    nc.sync.dma_start(out=stat_in[layer][:], in_=sloc[:])
                    if cfg.USE_AR:
                        nc.gpsimd.collective_compute(
                            "AllReduce",
                            AluOp.add,
                            replica_groups=[[0, 1, 2, 3, 4, 5, 6, 7]],
                            ins=[stat_in[layer][:]],
                            outs=[stat_out[layer][:]],
                        )
                    else:
                        nc.sync.dma_start(out=stat_out[layer][:], in_=stat_in[layer][:])
                    sglob = small.tile([P, 2], f32, tag="sglob")
                    nc.sync.dma_start(out=sglob[:], in_=stat_out[layer][:])
                    nc.scalar.mul(
                        out=sglob[:], in_=sglob[:], mul=0.125 if cfg.USE_AR else 1.0
                    )
                    var = small.tile([P, 1], f32, tag="var")
                    nc.vector.tensor_tensor(
                        out=var[:], in0=sglob[:, 0:1], in1=sglob[:, 0:1], op=AluOp.mult
                    )
                    nc.vector.tensor_sub(out=var[:], in0=sglob[:, 1:2], in1=var[:])
                    rstd = small.tile([P, 1], f32, tag="rstd")
                    nc.scalar.activation(
                        out=rstd[:], in_=var[:], func=ActF.Sqrt, bias=eps_sb[:]
                    )
                    nc.vector.reciprocal(out=rstd[:], in_=rstd[:])
                    scal = small.tile([P, 1], f32, tag="scal")
                    nc.vector.tensor_tensor(
                        out=scal[:], in0=gb_sb[layer][0][:], in1=rstd[:], op=AluOp.mult
                    )
                    shif = small.tile([P, 1], f32, tag="shif")
                    nc.vector.tensor_tensor(
                        out=shif[:], in0=sglob[:, 0:1], in1=scal[:], op=AluOp.mult
                    )
                    nc.vector.tensor_sub(out=shif[:], in0=gb_sb[layer][1][:], in1=shif[:])
                    # ---- pass 2: BN + relu + transpose + write shard rows
                    for t in range(cfg.TILES):
                        valid = cfg.VALID_LAST if t == cfg.TILES - 1 else P
                        for bat in range(2):
                            hbn = p2p.tile([P, P], bf16, tag=f"hbn{bat}")
                            nc.scalar.activation(
                                out=hbn[:],
                                in_=hraw[bat][:, t * P : (t + 1) * P],
                                func=ActF.Relu,
                                bias=shif[:],
                                scale=scal[:],
                            )
                            t_ps = ps_t.tile([P, P], bf16, tag="tps")
                            nc.tensor.transpose(
                                out=t_ps[:], in_=hbn[:], identity=ident[:]
                            )
                            hrow = p2p.tile([P, P], bf16, tag=f"hrow{bat}")
                            nc.vector.tensor_copy(out=hrow[:], in_=t_ps[:])
                            nc.sync.dma_start(
                                out=shard_out[layer][
                                    t * P : t * P + valid,
                                    bat * FEAT : (bat + 1) * FEAT,
                                ],
                                in_=hrow[:valid, :],
                            )
                    if cfg.USE_AG:
                        nc.gpsimd.collective_compute(
                            "AllGather",
                            AluOp.bypass,
                            replica_groups=[[0, 1, 2, 3, 4, 5, 6, 7]],
                            ins=[shard_out[layer][:]],
                            outs=[htab[layer][:]],
                        )
                    else:
                        nc.sync.dma_start(
                            out=htab[layer][0 : cfg.SHARD, :], in_=shard_out[layer][:]
                        )
                else:
                    nfull = cfg.TILES - 1
                    for bat, od_ in ((0, out0_d), (1, out1_d)):
                        if nfull:
                            nc.sync.dma_start(
                                out=od_[0 : nfull * P].rearrange("(t p) -> p t", p=P),
                                in_=out_sb[bat][:, 0:nfull],
                            )
                        nc.sync.dma_start(
                            out=od_[nfull * P : cfg.SHARD, None],
                            in_=out_sb[bat][: cfg.VALID_LAST, nfull : nfull + 1],
                        )

    nc.compile()
    return nc


# ---------------------------------------------------------------- consts + run


def _const_inputs():
    import jax.numpy as jnp

    iota_w = np.tile(np.arange(WIN, dtype=np.float32), (P, 1))
    iota_p = np.tile(np.arange(P, dtype=np.float32), (P, 1))
    ident = np.asarray(jnp.asarray(np.eye(P, dtype=np.float32), dtype=jnp.bfloat16))
    return iota_w, iota_p, ident


def run_gcn(cfg, inputs, trace=False):
    import jax.numpy as jnp

    from concourse.bass_utils import run_bass_kernel_spmd

    def bf(a):
        return np.asarray(jnp.asarray(np.asarray(a, np.float32), dtype=jnp.bfloat16))

    x = np.asarray(inputs["x"], dtype=np.float32)
    edge_index = np.asarray(inputs["edge_index"])
    per_shard, OVC, OG = build_schedule(cfg, edge_index)
    shapes = {k: v.shape for k, v in per_shard[0].items()}
    nc = build_nc(cfg, OVC, OG, shapes)

    # merged bf16 node table: row n = [x0[n](128) | x1[n](128)]
    xm = np.concatenate([x[0], x[1]], axis=1)
    xm = bf(xm)

    iota_w, iota_p, ident = _const_inputs()
    common = {
        "xm_tab": xm,
        "W1": bf(inputs["W1"]),
        "W2": bf(inputs["W2"]),
        "W3": bf(inputs["W3"]),
        "b1": np.asarray(inputs["b1"], np.float32),
        "b2": np.asarray(inputs["b2"], np.float32),
        "b3": np.asarray(inputs["b3"], np.float32),
        "gamma1": np.asarray(inputs["gamma1"], np.float32),
        "beta1": np.asarray(inputs["beta1"], np.float32),
        "gamma2": np.asarray(inputs["gamma2"], np.float32),
        "beta2": np.asarray(inputs["beta2"], np.float32),
        "iota_w": iota_w,
        "iota_p": iota_p,
        "ident": ident,
    }
    in_maps = []
    for c in range(NSHARD):
        m = dict(common)
        m.update(per_shard[c])
        in_maps.append(m)

    try:
        res = run_bass_kernel_spmd(nc, in_maps, core_ids=list(range(8)), trace=trace)
    except ModuleNotFoundError:
        res = run_bass_kernel_spmd(nc, in_maps, core_ids=list(range(8)), trace=False)
    out = np.empty((cfg.BATCH, cfg.N), np.float32)
    for c in range(NSHARD):
        out[0, c * cfg.SHARD : (c + 1) * cfg.SHARD] = res.results[c]["out0"]
        out[1, c * cfg.SHARD : (c + 1) * cfg.SHARD] = res.results[c]["out1"]
    return out, res


def kernel(**inputs) -> np.ndarray:
    cfg = Cfg()
    out, _ = run_gcn(cfg, inputs, trace=False)
    return out


# revision 8
# speedup vs baseline: 1.8941x; 1.0038x over previous
"""ClusterGCN (3-layer GCN, sum-aggregation) on 8 Trainium2 NeuronCores.

Strategy (hardcoded for B=2, N=50000, F=H=128, E=800000, 8 cores):
  - Batch-merged tables: node row = [h_b0(128) | h_b1(128)] bf16 (512 B), so
    ONE dma_gather descriptor per edge serves both batch elements. SWDGE
    descriptor generation on GpSimd (~8 ns/row) is the bottleneck; halving
    descriptors nearly halves the kernel.
  - core c owns destination nodes [c*6250, (c+1)*6250) for BOTH batches.
  - Reassociate each layer: A @ (h @ W) == (A @ h) @ W, so every layer is
    gather-aggregate (segment-sum over edges) followed by a dense 128x128
    matmul per batch. Aggregation output lives as agg_T[f, d] bf16 matmuls
    accumulating into two PSUM tiles (one per batch).
  - Per-edge gathers use the SWDGE dma_gather custom instruction (int16
    indices, max 1024 per call). Indices only reach 32767 rows, so each edge
    stream is split into a low (src < 25000) and high (src >= 25000) bucket
    gathered from offset views of the table.
  - Segment-sum on the tensor engine: edges are grouped per 16-destination
    window into 128-slot chunks; chunk x onehot(dest_rel) matmuls accumulate
    into PSUM tiles of 128 destinations. Window overflow edges go to
    per-tile overflow chunks gathered in batched calls across tile groups.
  - BatchNorm is training-mode over all B*N rows: per-core bn_stats/bn_aggr,
    then an 8-core AllReduce of (mean, E[x^2]).
  - After BN+ReLU the shard rows are transposed back to row-major bf16 and
    AllGathered across all 8 cores into the next gather table.
"""

import math

import numpy as np

# ---------------------------------------------------------------- config

P = 128
FEAT = 128  # per-batch feature width
TW = 256  # merged table row width (2 batches)
WIN = 16
WPT = P // WIN  # windows per tile (8); one window call = WPT*128 = 1024 idx
NSHARD = 8


class Cfg:
    def __init__(self, n_nodes=50000, batch=2, eps=1e-5):
        assert n_nodes % NSHARD == 0
        self.N = n_nodes
        self.SHARD = n_nodes // NSHARD
        self.BATCH = batch
        self.HALF = n_nodes // 2
        assert self.HALF <= 32767
        self.TILES = math.ceil(self.SHARD / P)
        self.VALID_LAST = self.SHARD - (self.TILES - 1) * P
        self.EPS = eps
        # debug toggles
        self.LAYERS = 3
        self.USE_AR = True
        self.USE_AG = True


# ---------------------------------------------------------------- host schedule


def _shard_schedule(cfg, row, col, q):
    """Per-shard edge schedule: window slots + overflow lists.

    Returns (win_idx [NW,2,128] int16, win_drel [NW,2,128] f32,
             ov: dict[(tile, bucket)] -> (idx16 1d, drel 1d))."""
    base = q * cfg.SHARD
    m = (col >= base) & (col < base + cfg.SHARD)
    r = row[m].astype(np.int64)
    c = (col[m] - base).astype(np.int64)
    wg = c // WIN
    bkt = (r >= cfg.HALF).astype(np.int64)
    key = wg * 2 + bkt
    order = np.argsort(key, kind="stable")
    r, c, wg, bkt, key = r[order], c[order], wg[order], bkt[order], key[order]
    n = len(key)
    NW = cfg.TILES * WPT

    if n == 0:
        return (
            np.zeros((NW, 2, P), np.int16),
            np.full((NW, 2, P), 255.0, np.float32),
            {},
        )

    newg = np.empty(n, bool)
    newg[0] = True
    newg[1:] = key[1:] != key[:-1]
    gstart = np.flatnonzero(newg)
    counts = np.diff(np.append(gstart, n))
    starts = np.repeat(gstart, counts)
    pos = np.arange(n) - starts
    idx16 = np.where(bkt == 1, r - cfg.HALF, r).astype(np.int16)

    inw = pos < P
    win_idx = np.zeros((NW, 2, P), np.int16)
    win_drel = np.full((NW, 2, P), 255.0, np.float32)
    win_idx[wg[inw], bkt[inw], pos[inw]] = idx16[inw]
    win_drel[wg[inw], bkt[inw], pos[inw]] = (c[inw] - wg[inw] * WIN).astype(np.float32)

    ov = {}
    ow = ~inw
    if ow.any():
        t_ov = wg[ow] // WPT
        b_ov = bkt[ow]
        i_ov = idx16[ow]
        d_ov = (c[ow] - t_ov * P).astype(np.float32)
        okey = t_ov * 2 + b_ov
        oorder = np.argsort(okey, kind="stable")
        t_ov, b_ov, i_ov, d_ov, okey = (
            t_ov[oorder],
            b_ov[oorder],
            i_ov[oorder],
            d_ov[oorder],
            okey[oorder],
        )
        bounds = np.flatnonzero(np.append(True, okey[1:] != okey[:-1]))
        bounds = np.append(bounds, len(okey))
        for j in range(len(bounds) - 1):
            s, e = bounds[j], bounds[j + 1]
            ov[(int(t_ov[s]), int(b_ov[s]))] = (i_ov[s:e], d_ov[s:e])
    return win_idx, win_drel, ov


def _wrap16(stream):
    """[n] idx stream -> [128, n/16] wrapped col-major, replicated x8."""
    return np.tile(stream.reshape(-1, 16).T, (8, 1))


def build_schedule(cfg, edge_index):
    """Build gather-index / dest-rel input tensors for the 8 shards.

    Layout (layer-invariant, loaded once):
      win_idx  [128, TILES*128] i16 : tile t -> cols [t*128, t*128+64) = lo
               window stream (8 chunks), [+64, +128) = hi stream.
      win_drel [128, TILES*16] f32  : tile t -> cols [t*16+w] lo, [t*16+8+w] hi.
      ov_idx   [128, NOVG*OG*OVC*8*2] i16 : group og -> lo block then hi block.
      ov_drel  [128, NOVG*OG*OVC*2] f32   : group og -> lo cols then hi cols.

    Returns (per_shard list of dicts, OVC, OG)."""
    row = np.asarray(edge_index[0])
    col = np.asarray(edge_index[1])
    shards = [_shard_schedule(cfg, row, col, q) for q in range(NSHARD)]

    ovc = 0
    for _, _, ov in shards:
        for (t, b), (i1, _) in ov.items():
            ovc = max(ovc, math.ceil(len(i1) / P))
    OVC = ovc
    OG = max(1, WPT // OVC) if OVC else 1
    assert OG * OVC <= WPT, f"overflow call too large: OG={OG} OVC={OVC}"
    NOVG = math.ceil(cfg.TILES / OG)

    per_shard = []
    for win_idx, win_drel, ov in shards:
        wi = np.zeros((128, cfg.TILES * P), np.int16)
        wd = np.full((128, cfg.TILES * WIN), 255.0, np.float32)
        for t in range(cfg.TILES):
            for b in (0, 1):
                stream = np.concatenate(
                    [win_idx[t * WPT + w, b] for w in range(WPT)]
                )
                wi[:, t * P + b * 64 : t * P + (b + 1) * 64] = _wrap16(stream)
                for w in range(WPT):
                    wd[:, t * WIN + b * WPT + w] = win_drel[t * WPT + w, b]
        if OVC:
            oi = np.zeros((128, NOVG * OG * OVC * 8 * 2), np.int16)
            od = np.full((128, NOVG * OG * OVC * 2), 255.0, np.float32)
            for og in range(NOVG):
                t0, t1 = og * OG, min((og + 1) * OG, cfg.TILES)
                ogg = t1 - t0
                base_i = og * OG * OVC * 8 * 2
                base_d = og * OG * OVC * 2
                for b in (0, 1):
                    chunks = []
                    for tl, t in enumerate(range(t0, t1)):
                        e_i, e_d = ov.get(
                            (t, b), (np.zeros(0, np.int16), np.zeros(0, np.float32))
                        )
                        cap = OVC * P
                        pi = np.zeros(cap, np.int16)
                        pd = np.full(cap, 255.0, np.float32)
                        pi[: len(e_i)] = e_i
                        pd[: len(e_d)] = e_d
                        chunks.append(pi)
                        for j in range(OVC):
                            od[:, base_d + b * ogg * OVC + tl * OVC + j] = pd[
                                j * P : (j + 1) * P
                            ]
                    stream = np.concatenate(chunks)
                    blk = _wrap16(stream)
                    off = base_i + b * ogg * OVC * 8
                    oi[:, off : off + blk.shape[1]] = blk
        else:
            oi = np.zeros((128, 16), np.int16)
            od = np.full((128, 2), 255.0, np.float32)
        per_shard.append(dict(win_idx=wi, win_drel=wd, ov_idx=oi, ov_drel=od))
    return per_shard, OVC, OG


# ---------------------------------------------------------------- bass kernel


def build_nc(cfg, OVC, OG, shapes):
    import concourse.bacc as bacc
    import concourse.bass as bass
    import concourse.tile as tile
    from concourse import mybir

    f32 = mybir.dt.float32
    bf16 = mybir.dt.float16
    i16 = mybir.dt.int16
    NOVG = math.ceil(cfg.TILES / OG)

    nc = bacc.Bacc("TRN2", target_bir_lowering=False, debug=False, num_devices=8)

    # merged bf16 node table for layer 0 (built on host from x)
    xm_d = nc.dram_tensor("xm_tab", [cfg.N, TW], bf16, kind="ExternalInput")
    wi_d = nc.dram_tensor("win_idx", list(shapes["win_idx"]), i16, kind="ExternalInput")
    wd_d = nc.dram_tensor("win_drel", list(shapes["win_drel"]), f32, kind="ExternalInput")
    oi_d = nc.dram_tensor("ov_idx", list(shapes["ov_idx"]), i16, kind="ExternalInput")
    od_d = nc.dram_tensor("ov_drel", list(shapes["ov_drel"]), f32, kind="ExternalInput")
    w1_d = nc.dram_tensor("W1", [FEAT, FEAT], bf16, kind="ExternalInput")
    w2_d = nc.dram_tensor("W2", [FEAT, FEAT], bf16, kind="ExternalInput")
    w3_d = nc.dram_tensor("W3", [FEAT, 1], bf16, kind="ExternalInput")
    b1_d = nc.dram_tensor("b1", [FEAT], f32, kind="ExternalInput")
    b2_d = nc.dram_tensor("b2", [FEAT], f32, kind="ExternalInput")
    b3_d = nc.dram_tensor("b3", [1], f32, kind="ExternalInput")
    gam1_d = nc.dram_tensor("gamma1", [FEAT], f32, kind="ExternalInput")
    bet1_d = nc.dram_tensor("beta1", [FEAT], f32, kind="ExternalInput")
    gam2_d = nc.dram_tensor("gamma2", [FEAT], f32, kind="ExternalInput")
    bet2_d = nc.dram_tensor("beta2", [FEAT], f32, kind="ExternalInput")
    iota_w_d = nc.dram_tensor("iota_w", [P, WIN], f32, kind="ExternalInput")
    iota_p_d = nc.dram_tensor("iota_p", [P, P], f32, kind="ExternalInput")
    ident_d = nc.dram_tensor("ident", [P, P], bf16, kind="ExternalInput")
    out0_d = nc.dram_tensor("out0", [cfg.SHARD], f32, kind="ExternalOutput")
    out1_d = nc.dram_tensor("out1", [cfg.SHARD], f32, kind="ExternalOutput")

    htab = [
        nc.dram_tensor(f"htab{i}", [cfg.N, TW], bf16, kind="Internal")
        for i in range(2)
    ]
    shard_out = [
        nc.dram_tensor(f"shard_out{i}", [cfg.SHARD, TW], bf16, kind="Internal")
        for i in range(2)
    ]
    stat_in = [
        nc.dram_tensor(f"stat_in{i}", [P, 2], f32, kind="Internal") for i in range(2)
    ]
    stat_out = [
        nc.dram_tensor(f"stat_out{i}", [P, 2], f32, kind="Internal") for i in range(2)
    ]

    AluOp = mybir.AluOpType
    ActF = mybir.ActivationFunctionType

    def bcast_inner(ap, inner):
        """ap [128, k] -> [128, k, inner] with 0-stride inner axis."""
        return bass.AP(
            tensor=ap.tensor,
            offset=ap.offset,
            ap=[list(ap.ap[0]), list(ap.ap[1]), [0, inner]],
        )

    def bcast_rep(ap, reps):
        """ap [128, k] -> [128, reps, k] with 0-stride middle axis."""
        return bass.AP(
            tensor=ap.tensor,
            offset=ap.offset,
            ap=[list(ap.ap[0]), [0, reps], list(ap.ap[1])],
        )

    with tile.TileContext(nc) as tc:
        with (
            tc.tile_pool(name="consts", bufs=1) as consts,
            tc.tile_pool(name="gwin", bufs=6) as gwinp,
            tc.tile_pool(name="gov", bufs=3) as govp,
            tc.tile_pool(name="ohp", bufs=4) as ohp,
            tc.tile_pool(name="aggp", bufs=4) as aggp,
            tc.tile_pool(name="hraw", bufs=1) as hrawp,
            tc.tile_pool(name="statp", bufs=2) as statp,
            tc.tile_pool(name="small", bufs=8) as small,
            tc.tile_pool(name="p2", bufs=3) as p2p,
            tc.tile_pool(name="outp", bufs=1) as outp,
            tc.tile_pool(name="ps_agg", bufs=2, space="PSUM") as ps_agg,
            tc.tile_pool(name="ps_h", bufs=2, space="PSUM") as ps_h,
            tc.tile_pool(name="ps_t", bufs=2, space="PSUM") as ps_t,
        ):
            # ---- layer-invariant inputs (indices, dest_rel, weights, consts)
            wi_sb = consts.tile(list(shapes["win_idx"]), i16, tag="wi")
            nc.sync.dma_start(out=wi_sb[:], in_=wi_d[:])
            wd_sb = consts.tile(list(shapes["win_drel"]), f32, tag="wd")
            nc.sync.dma_start(out=wd_sb[:], in_=wd_d[:])
            oi_sb = consts.tile(list(shapes["ov_idx"]), i16, tag="oi")
            nc.sync.dma_start(out=oi_sb[:], in_=oi_d[:])
            od_sb = consts.tile(list(shapes["ov_drel"]), f32, tag="od")
            nc.sync.dma_start(out=od_sb[:], in_=od_d[:])

            w_sb = []
            for wdr in (w1_d, w2_d):
                t = consts.tile([P, FEAT], bf16, tag=f"w_{wdr.name}")
                nc.sync.dma_start(out=t[:], in_=wdr[:])
                w_sb.append(t)
            w3_sb = consts.tile([P, 1], bf16, tag="w3")
            nc.sync.dma_start(out=w3_sb[:], in_=w3_d[:])
            b_sb = []
            for bd in (b1_d, b2_d):
                t = consts.tile([P, 1], f32, tag=f"b_{bd.name}")
                nc.sync.dma_start(out=t[:], in_=bd[:, None])
                b_sb.append(t)
            b3_sb = consts.tile([P, 1], f32, tag="b3")
            nc.sync.dma_start(out=b3_sb[:], in_=b3_d[:].to_broadcast([P, 1]))
            gb_sb = []
            for gd, bd in ((gam1_d, bet1_d), (gam2_d, bet2_d)):
                tg_ = consts.tile([P, 1], f32, tag=f"g_{gd.name}")
                nc.sync.dma_start(out=tg_[:], in_=gd[:, None])
                tb_ = consts.tile([P, 1], f32, tag=f"be_{bd.name}")
                nc.sync.dma_start(out=tb_[:], in_=bd[:, None])
                gb_sb.append((tg_, tb_))
            iota_w = consts.tile([P, WIN], f32, tag="iota_w")
            nc.sync.dma_start(out=iota_w[:], in_=iota_w_d[:])
            iota_p = consts.tile([P, P], f32, tag="iota_p")
            nc.sync.dma_start(out=iota_p[:], in_=iota_p_d[:])
            ident = consts.tile([P, P], bf16, tag="ident")
            nc.sync.dma_start(out=ident[:], in_=ident_d[:])
            eps_sb = consts.tile([P, 1], f32, tag="eps")
            nc.vector.memset(eps_sb[:], cfg.EPS)

            for layer in range(cfg.LAYERS):
                table = xm_d if layer == 0 else htab[layer - 1]
                is_last = layer == cfg.LAYERS - 1
                if not is_last:
                    # hraw: [fout, dest] per batch, f32
                    hraw = [
                        hrawp.tile(
                            [P, cfg.TILES * P], f32, tag=f"hraw{bat}",
                            name=f"hraw{bat}",
                        )
                        for bat in range(2)
                    ]
                    stat_t = statp.tile([P, 2 * cfg.TILES, 6], f32, tag="stats")
                else:
                    out_sb = [
                        outp.tile(
                            [P, cfg.TILES], f32, tag=f"outsb{bat}",
                            name=f"outsb{bat}",
                        )
                        for bat in range(2)
                    ]

                for og in range(NOVG):
                    t0, t1 = og * OG, min((og + 1) * OG, cfg.TILES)
                    ogg = t1 - t0
                    gov = []
                    if OVC:
                        nch = ogg * OVC
                        base_i = og * OG * OVC * 8 * 2
                        for b in (0, 1):
                            gt = govp.tile([P, OG * OVC, TW], bf16, tag=f"gov{b}")
                            src = table[:, :] if b == 0 else table[cfg.HALF :, :]
                            nc.gpsimd.dma_gather(
                                gt[:, :nch, :],
                                src,
                                oi_sb[:, base_i + b * nch * 8 : base_i + (b + 1) * nch * 8],
                                nch * P,
                                nch * P,
                                TW,
                            )
                            gov.append(gt)

                    for t in range(t0, t1):
                        tl = t - t0
                        gwin = []
                        for b in (0, 1):
                            gt = gwinp.tile([P, WPT, TW], bf16, tag=f"gwin{b}")
                            src = table[:, :] if b == 0 else table[cfg.HALF :, :]
                            nc.gpsimd.dma_gather(
                                gt[:],
                                src,
                                wi_sb[:, t * P + b * 64 : t * P + (b + 1) * 64],
                                WPT * P,
                                WPT * P,
                                TW,
                            )
                            gwin.append(gt)
                        oh_w = []
                        oh_o = []
                        for b in (0, 1):
                            t_ohw = ohp.tile([P, WPT * WIN], bf16, tag=f"ohw{b}")
                            nc.vector.tensor_tensor(
                                out=t_ohw[:],
                                in0=bcast_inner(
                                    wd_sb[:, t * WIN + b * WPT : t * WIN + (b + 1) * WPT],
                                    WIN,
                                ),
                                in1=bcast_rep(iota_w[:], WPT),
                                op=AluOp.is_equal,
                            )
                            oh_w.append(t_ohw)
                            if OVC:
                                base_d = og * OG * OVC * 2
                                c0 = base_d + b * ogg * OVC + tl * OVC
                                t_oho = ohp.tile([P, OVC * P], bf16, tag=f"oho{b}")
                                nc.vector.tensor_tensor(
                                    out=t_oho[:],
                                    in0=bcast_inner(od_sb[:, c0 : c0 + OVC], P),
                                    in1=bcast_rep(iota_p[:], OVC),
                                    op=AluOp.is_equal,
                                )
                                oh_o.append(t_oho)

                        # two PSUM accumulators, one per batch element
                        agg_ps = [
                            ps_agg.tile(
                                [P, P], f32, tag=f"agg{bat}", name=f"agg{bat}"
                            )
                            for bat in range(2)
                        ]
                        n_ov = 2 * OVC
                        for bat in range(2):
                            fsl = slice(bat * FEAT, (bat + 1) * FEAT)
                            for b in (0, 1):
                                for w in range(WPT):
                                    nc.tensor.matmul(
                                        agg_ps[bat][:, w * WIN : (w + 1) * WIN],
                                        lhsT=gwin[b][:, w, fsl],
                                        rhs=oh_w[b][:, w * WIN : (w + 1) * WIN],
                                        start=(b == 0 and w == 0),
                                        stop=(n_ov == 0 and b == 1 and w == WPT - 1),
                                    )
                            k_ov = 0
                            for b in range(2):
                                for j in range(OVC):
                                    k_ov += 1
                                    nc.tensor.matmul(
                                        agg_ps[bat][:, :],
                                        lhsT=gov[b][:, tl * OVC + j, fsl],
                                        rhs=oh_o[b][:, j * P : (j + 1) * P],
                                        start=False,
                                        stop=(k_ov == n_ov),
                                    )

                        valid = cfg.VALID_LAST if t == cfg.TILES - 1 else P
                        for bat in range(2):
                            agg_sb = aggp.tile([P, P], bf16, tag=f"aggsb{bat}")
                            nc.vector.tensor_copy(out=agg_sb[:], in_=agg_ps[bat][:])
                            if not is_last:
                                h_ps = ps_h.tile([P, P], f32, tag="hps")
                                nc.tensor.matmul(
                                    h_ps[:], lhsT=w_sb[layer][:], rhs=agg_sb[:],
                                    start=True, stop=True,
                                )
                                nc.vector.tensor_scalar_add(
                                    out=hraw[bat][:, t * P : t * P + P],
                                    in0=h_ps[:],
                                    scalar1=b_sb[layer][:],
                                )
                                nc.vector.bn_stats(
                                    out=stat_t[:, 2 * t + bat, :],
                                    in_=hraw[bat][:, t * P : t * P + valid],
                                )
                            else:
                                o_ps = ps_h.tile([P, 1], f32, tag="hps")
                                nc.tensor.matmul(
                                    o_ps[:], lhsT=agg_sb[:], rhs=w3_sb[:],
                                    start=True, stop=True,
                                )
                                nc.vector.tensor_scalar_add(
                                    out=out_sb[bat][:, t : t + 1],
                                    in0=o_ps[:],
                                    scalar1=b3_sb[:],
                                )

                if not is_last:
                    # ---- global BN stats
                    mv = small.tile([P, 2], f32, tag="mv")
                    nc.vector.bn_aggr(out=mv[:], in_=stat_t[:, :, :])
                    sloc = small.tile([P, 2], f32, tag="sloc")
                    nc.vector.tensor_copy(out=sloc[:, 0:1], in_=mv[:, 0:1])
                    nc.vector.tensor_tensor(
                        out=sloc[:, 1:2], in0=mv[:, 0:1], in1=mv[:, 0:1], op=AluOp.mult
                    )
                    nc.vector.tensor_add(
                        out=sloc[:, 1:2], in0=sloc[:, 1:2], in1=mv[:, 1:2]
                    )
                    nc.sync.dma_start(out=stat_in[layer][:], in_=sloc[:])
                    if cfg.USE_AR:
                        nc.gpsimd.collective_compute(
                            "AllReduce",
                            AluOp.add,
                            replica_groups=[[0, 1, 2, 3, 4, 5, 6, 7]],
                            ins=[stat_in[layer][:]],
                            outs=[stat_out[layer][:]],
                        )
                    else:
                        nc.sync.dma_start(out=stat_out[layer][:], in_=stat_in[layer][:])
                    sglob = small.tile([P, 2], f32, tag="sglob")
                    nc.sync.dma_start(out=sglob[:], in_=stat_out[layer][:])
                    nc.scalar.mul(
                        out=sglob[:], in_=sglob[:], mul=0.125 if cfg.USE_AR else 1.0
                    )
                    var = small.tile([P, 1], f32, tag="var")
                    nc.vector.tensor_tensor(
                        out=var[:], in0=sglob[:, 0:1], in1=sglob[:, 0:1], op=AluOp.mult
                    )
                    nc.vector.tensor_sub(out=var[:], in0=sglob[:, 1:2], in1=var[:])
                    rstd = small.tile([P, 1], f32, tag="rstd")
                    nc.scalar.activation(
                        out=rstd[:], in_=var[:], func=ActF.Sqrt, bias=eps_sb[:]
                    )
                    nc.vector.reciprocal(out=rstd[:], in_=rstd[:])
                    scal = small.tile([P, 1], f32, tag="scal")
                    nc.vector.tensor_tensor(
                        out=scal[:], in0=gb_sb[layer][0][:], in1=rstd[:], op=AluOp.mult
                    )
                    shif = small.tile([P, 1], f32, tag="shif")
                    nc.vector.tensor_tensor(
                        out=shif[:], in0=sglob[:, 0:1], in1=scal[:], op=AluOp.mult
                    )
                    nc.vector.tensor_sub(out=shif[:], in0=gb_sb[layer][1][:], in1=shif[:])
                    # ---- pass 2: BN + relu + transpose + write shard rows
                    for t in range(cfg.TILES):
                        valid = cfg.VALID_LAST if t == cfg.TILES - 1 else P
                        for bat in range(2):
                            hbn = p2p.tile([P, P], bf16, tag=f"hbn{bat}")
                            nc.scalar.activation(
                                out=hbn[:],
                                in_=hraw[bat][:, t * P : (t + 1) * P],
                                func=ActF.Relu,
                                bias=shif[:],
                                scale=scal[:],
                            )
                            t_ps = ps_t.tile([P, P], bf16, tag="tps")
                            nc.tensor.transpose(
                                out=t_ps[:], in_=hbn[:], identity=ident[:]
                            )
                            hrow = p2p.tile([P, P], bf16, tag=f"hrow{bat}")
                            nc.vector.tensor_copy(out=hrow[:], in_=t_ps[:])
                            nc.sync.dma_start(
                                out=shard_out[layer][
                                    t * P : t * P + valid,
                                    bat * FEAT : (bat + 1) * FEAT,
                                ],
                                in_=hrow[:valid, :],
                            )
                    if cfg.USE_AG:
                        nc.gpsimd.collective_compute(
                            "AllGather",
                            AluOp.bypass,
                            replica_groups=[[0, 1, 2, 3, 4, 5, 6, 7]],
                            ins=[shard_out[layer][:]],
                            outs=[htab[layer][:]],
                        )
                    else:
                        nc.sync.dma_start(
                            out=htab[layer][0 : cfg.SHARD, :], in_=shard_out[layer][:]
                        )
                else:
                    nfull = cfg.TILES - 1
                    for bat, od_ in ((0, out0_d), (1, out1_d)):
                        if nfull:
                            nc.sync.dma_start(
                                out=od_[0 : nfull * P].rearrange("(t p) -> p t", p=P),
                                in_=out_sb[bat][:, 0:nfull],
                            )
                        nc.sync.dma_start(
                            out=od_[nfull * P : cfg.SHARD, None],
                            in_=out_sb[bat][: cfg.VALID_LAST, nfull : nfull + 1],
                        )

    nc.compile()
    return nc


# ---------------------------------------------------------------- consts + run


def _const_inputs():
    import jax.numpy as jnp

    iota_w = np.tile(np.arange(WIN, dtype=np.float32), (P, 1))
    iota_p = np.tile(np.arange(P, dtype=np.float32), (P, 1))
    ident = np.asarray(jnp.asarray(np.eye(P, dtype=np.float32), dtype=jnp.float16))
    return iota_w, iota_p, ident


def run_gcn(cfg, inputs, trace=False):
    import jax.numpy as jnp

    from concourse.bass_utils import run_bass_kernel_spmd

    def bf(a):
        return np.asarray(jnp.asarray(np.asarray(a, np.float32), dtype=jnp.float16))

    x = np.asarray(inputs["x"], dtype=np.float32)
    edge_index = np.asarray(inputs["edge_index"])
    per_shard, OVC, OG = build_schedule(cfg, edge_index)
    shapes = {k: v.shape for k, v in per_shard[0].items()}
    nc = build_nc(cfg, OVC, OG, shapes)

    # merged bf16 node table: row n = [x0[n](128) | x1[n](128)]
    xm = np.concatenate([x[0], x[1]], axis=1)
    xm = bf(xm)

    iota_w, iota_p, ident = _const_inputs()
    common = {
        "xm_tab": xm,
        "W1": bf(inputs["W1"]),
        "W2": bf(inputs["W2"]),
        "W3": bf(inputs["W3"]),
        "b1": np.asarray(inputs["b1"], np.float32),
        "b2": np.asarray(inputs["b2"], np.float32),
        "b3": np.asarray(inputs["b3"], np.float32),
        "gamma1": np.asarray(inputs["gamma1"], np.float32),
        "beta1": np.asarray(inputs["beta1"], np.float32),
        "gamma2": np.asarray(inputs["gamma2"], np.float32),
        "beta2": np.asarray(inputs["beta2"], np.float32),
        "iota_w": iota_w,
        "iota_p": iota_p,
        "ident": ident,
    }
    in_maps = []
    for c in range(NSHARD):
        m = dict(common)
        m.update(per_shard[c])
        in_maps.append(m)

    try:
        res = run_bass_kernel_spmd(nc, in_maps, core_ids=list(range(8)), trace=trace)
    except ModuleNotFoundError:
        res = run_bass_kernel_spmd(nc, in_maps, core_ids=list(range(8)), trace=False)
    out = np.empty((cfg.BATCH, cfg.N), np.float32)
    for c in range(NSHARD):
        out[0, c * cfg.SHARD : (c + 1) * cfg.SHARD] = res.results[c]["out0"]
        out[1, c * cfg.SHARD : (c + 1) * cfg.SHARD] = res.results[c]["out1"]
    return out, res


def kernel(**inputs) -> np.ndarray:
    cfg = Cfg()
    out, _ = run_gcn(cfg, inputs, trace=False)
    return out


# revision 9
# speedup vs baseline: 1.9046x; 1.0056x over previous
"""ClusterGCN (3-layer GCN, sum-aggregation) on 8 Trainium2 NeuronCores.

Strategy (hardcoded for B=2, N=50000, F=H=128, E=800000, 8 cores):
  - Batch-merged tables: node row = [h_b0(128) | h_b1(128)] fp16 (512 B), so
    ONE dma_gather descriptor per edge serves both batch elements. SWDGE
    descriptor generation on GpSimd (~8 ns/row) is the bottleneck; halving
    descriptors nearly halves the kernel.
  - core c owns destination nodes [c*6250, (c+1)*6250) for BOTH batches.
  - Reassociate each layer: A @ (h @ W) == (A @ h) @ W, so every layer is
    gather-aggregate (segment-sum over edges) followed by a dense 128x128
    matmul per batch. Aggregation output lives as agg_T[f, d]; fp16 matmuls
    accumulating into two PSUM tiles (one per batch).
  - Per-edge gathers use the SWDGE dma_gather custom instruction (int16
    indices, max 1024 per call). Indices only reach 32767 rows, so each edge
    stream is split into a low (src < 25000) and high (src >= 25000) bucket
    gathered from offset views of the table.
  - Segment-sum on the tensor engine: edges are grouped per 16-destination
    window into 128-slot chunks; chunk x onehot(dest_rel) matmuls accumulate
    into PSUM tiles of 128 destinations. Window overflow edges go to
    per-tile overflow chunks gathered in batched calls across tile groups.
  - BatchNorm is training-mode over all B*N rows: per-core bn_stats/bn_aggr,
    then an 8-core AllReduce of (mean, E[x^2]).
  - After BN+ReLU the shard rows are transposed back to row-major fp16 and
    AllGathered across all 8 cores into the next gather table.
"""

import math

import numpy as np

# ---------------------------------------------------------------- config

P = 128
FEAT = 128  # per-batch feature width
TW = 256  # merged table row width (2 batches)
WIN = 16
WPT = P // WIN  # windows per tile (8); one window call = WPT*128 = 1024 idx
NSHARD = 8


class Cfg:
    def __init__(self, n_nodes=50000, batch=2, eps=1e-5):
        assert n_nodes % NSHARD == 0
        self.N = n_nodes
        self.SHARD = n_nodes // NSHARD
        self.BATCH = batch
        self.HALF = n_nodes // 2
        assert self.HALF <= 32767
        self.TILES = math.ceil(self.SHARD / P)
        self.VALID_LAST = self.SHARD - (self.TILES - 1) * P
        self.EPS = eps
        # debug toggles
        self.LAYERS = 3
        self.USE_AR = True
        self.USE_AG = True


# ---------------------------------------------------------------- host schedule


def _shard_schedule(cfg, row, col, q):
    """Per-shard edge schedule: window slots + overflow lists.

    Returns (win_idx [NW,2,128] int16, win_drel [NW,2,128] f32,
             ov: dict[(tile, bucket)] -> (idx16 1d, drel 1d))."""
    base = q * cfg.SHARD
    m = (col >= base) & (col < base + cfg.SHARD)
    r = row[m].astype(np.int64)
    c = (col[m] - base).astype(np.int64)
    wg = c // WIN
    bkt = (r >= cfg.HALF).astype(np.int64)
    key = wg * 2 + bkt
    order = np.argsort(key, kind="stable")
    r, c, wg, bkt, key = r[order], c[order], wg[order], bkt[order], key[order]
    n = len(key)
    NW = cfg.TILES * WPT

    if n == 0:
        return (
            np.zeros((NW, 2, P), np.int16),
            np.full((NW, 2, P), 255.0, np.float32),
            {},
        )

    newg = np.empty(n, bool)
    newg[0] = True
    newg[1:] = key[1:] != key[:-1]
    gstart = np.flatnonzero(newg)
    counts = np.diff(np.append(gstart, n))
    starts = np.repeat(gstart, counts)
    pos = np.arange(n) - starts
    idx16 = np.where(bkt == 1, r - cfg.HALF, r).astype(np.int16)

    inw = pos < P
    win_idx = np.zeros((NW, 2, P), np.int16)
    win_drel = np.full((NW, 2, P), 255.0, np.float32)
    win_idx[wg[inw], bkt[inw], pos[inw]] = idx16[inw]
    win_drel[wg[inw], bkt[inw], pos[inw]] = (c[inw] - wg[inw] * WIN).astype(np.float32)

    ov = {}
    ow = ~inw
    if ow.any():
        t_ov = wg[ow] // WPT
        b_ov = bkt[ow]
        i_ov = idx16[ow]
        d_ov = (c[ow] - t_ov * P).astype(np.float32)
        okey = t_ov * 2 + b_ov
        oorder = np.argsort(okey, kind="stable")
        t_ov, b_ov, i_ov, d_ov, okey = (
            t_ov[oorder],
            b_ov[oorder],
            i_ov[oorder],
            d_ov[oorder],
            okey[oorder],
        )
        bounds = np.flatnonzero(np.append(True, okey[1:] != okey[:-1]))
        bounds = np.append(bounds, len(okey))
        for j in range(len(bounds) - 1):
            s, e = bounds[j], bounds[j + 1]
            ov[(int(t_ov[s]), int(b_ov[s]))] = (i_ov[s:e], d_ov[s:e])
    return win_idx, win_drel, ov


def _wrap16(stream):
    """[n] idx stream -> [128, n/16] wrapped col-major, replicated x8."""
    return np.tile(stream.reshape(-1, 16).T, (8, 1))


def build_schedule(cfg, edge_index):
    """Build gather-index / dest-rel input tensors for the 8 shards.

    Layout (layer-invariant, loaded once):
      win_idx  [128, TILES*128] i16 : tile t -> cols [t*128, t*128+64) = lo
               window stream (8 chunks), [+64, +128) = hi stream.
      win_drel [128, TILES*16] f32  : tile t -> cols [t*16+w] lo, [t*16+8+w] hi.
      ov_idx   [128, NOVG*OG*OVC*8*2] i16 : group og -> lo block then hi block.
      ov_drel  [128, NOVG*OG*OVC*2] f32   : group og -> lo cols then hi cols.

    Returns (per_shard list of dicts, OVC, OG)."""
    row = np.asarray(edge_index[0])
    col = np.asarray(edge_index[1])
    shards = [_shard_schedule(cfg, row, col, q) for q in range(NSHARD)]

    ovc = 0
    for _, _, ov in shards:
        for (t, b), (i1, _) in ov.items():
            ovc = max(ovc, math.ceil(len(i1) / P))
    OVC = ovc
    OG = max(1, WPT // OVC) if OVC else 1
    assert OG * OVC <= WPT, f"overflow call too large: OG={OG} OVC={OVC}"
    NOVG = math.ceil(cfg.TILES / OG)

    per_shard = []
    for win_idx, win_drel, ov in shards:
        wi = np.zeros((128, cfg.TILES * P), np.int16)
        wd = np.full((128, cfg.TILES * WIN), 255.0, np.float32)
        for t in range(cfg.TILES):
            for b in (0, 1):
                stream = np.concatenate(
                    [win_idx[t * WPT + w, b] for w in range(WPT)]
                )
                wi[:, t * P + b * 64 : t * P + (b + 1) * 64] = _wrap16(stream)
                for w in range(WPT):
                    wd[:, t * WIN + b * WPT + w] = win_drel[t * WPT + w, b]
        if OVC:
            oi = np.zeros((128, NOVG * OG * OVC * 8 * 2), np.int16)
            od = np.full((128, NOVG * OG * OVC * 2), 255.0, np.float32)
            for og in range(NOVG):
                t0, t1 = og * OG, min((og + 1) * OG, cfg.TILES)
                ogg = t1 - t0
                base_i = og * OG * OVC * 8 * 2
                base_d = og * OG * OVC * 2
                for b in (0, 1):
                    chunks = []
                    for tl, t in enumerate(range(t0, t1)):
                        e_i, e_d = ov.get(
                            (t, b), (np.zeros(0, np.int16), np.zeros(0, np.float32))
                        )
                        cap = OVC * P
                        pi = np.zeros(cap, np.int16)
                        pd = np.full(cap, 255.0, np.float32)
                        pi[: len(e_i)] = e_i
                        pd[: len(e_d)] = e_d
                        chunks.append(pi)
                        for j in range(OVC):
                            od[:, base_d + b * ogg * OVC + tl * OVC + j] = pd[
                                j * P : (j + 1) * P
                            ]
                    stream = np.concatenate(chunks)
                    blk = _wrap16(stream)
                    off = base_i + b * ogg * OVC * 8
                    oi[:, off : off + blk.shape[1]] = blk
        else:
            oi = np.zeros((128, 16), np.int16)
            od = np.full((128, 2), 255.0, np.float32)
        per_shard.append(dict(win_idx=wi, win_drel=wd, ov_idx=oi, ov_drel=od))
    return per_shard, OVC, OG


# ---------------------------------------------------------------- bass kernel


def build_nc(cfg, OVC, OG, shapes):
    import concourse.bacc as bacc
    import concourse.bass as bass
    import concourse.tile as tile
    from concourse import mybir

    f32 = mybir.dt.float32
    bf16 = mybir.dt.float16
    i16 = mybir.dt.int16
    NOVG = math.ceil(cfg.TILES / OG)

    nc = bacc.Bacc("TRN2", target_bir_lowering=False, debug=False, num_devices=8)

    # merged bf16 node table for layer 0 (built on host from x)
    xm_d = nc.dram_tensor("xm_tab", [cfg.N, TW], bf16, kind="ExternalInput")
    wi_d = nc.dram_tensor("win_idx", list(shapes["win_idx"]), i16, kind="ExternalInput")
    wd_d = nc.dram_tensor("win_drel", list(shapes["win_drel"]), f32, kind="ExternalInput")
    oi_d = nc.dram_tensor("ov_idx", list(shapes["ov_idx"]), i16, kind="ExternalInput")
    od_d = nc.dram_tensor("ov_drel", list(shapes["ov_drel"]), f32, kind="ExternalInput")
    w1_d = nc.dram_tensor("W1", [FEAT, FEAT], bf16, kind="ExternalInput")
    w2_d = nc.dram_tensor("W2", [FEAT, FEAT], bf16, kind="ExternalInput")
    w3_d = nc.dram_tensor("W3", [FEAT, 1], bf16, kind="ExternalInput")
    b1_d = nc.dram_tensor("b1", [FEAT], f32, kind="ExternalInput")
    b2_d = nc.dram_tensor("b2", [FEAT], f32, kind="ExternalInput")
    b3_d = nc.dram_tensor("b3", [1], f32, kind="ExternalInput")
    gam1_d = nc.dram_tensor("gamma1", [FEAT], f32, kind="ExternalInput")
    bet1_d = nc.dram_tensor("beta1", [FEAT], f32, kind="ExternalInput")
    gam2_d = nc.dram_tensor("gamma2", [FEAT], f32, kind="ExternalInput")
    bet2_d = nc.dram_tensor("beta2", [FEAT], f32, kind="ExternalInput")
    iota_w_d = nc.dram_tensor("iota_w", [P, WIN], f32, kind="ExternalInput")
    iota_p_d = nc.dram_tensor("iota_p", [P, P], f32, kind="ExternalInput")
    ident_d = nc.dram_tensor("ident", [P, P], bf16, kind="ExternalInput")
    out0_d = nc.dram_tensor("out0", [cfg.SHARD], f32, kind="ExternalOutput")
    out1_d = nc.dram_tensor("out1", [cfg.SHARD], f32, kind="ExternalOutput")

    htab = [
        nc.dram_tensor(f"htab{i}", [cfg.N, TW], bf16, kind="Internal")
        for i in range(2)
    ]
    shard_out = [
        nc.dram_tensor(f"shard_out{i}", [cfg.SHARD, TW], bf16, kind="Internal")
        for i in range(2)
    ]
    stat_in = [
        nc.dram_tensor(f"stat_in{i}", [P, 2], f32, kind="Internal") for i in range(2)
    ]
    stat_out = [
        nc.dram_tensor(f"stat_out{i}", [P, 2], f32, kind="Internal") for i in range(2)
    ]

    AluOp = mybir.AluOpType
    ActF = mybir.ActivationFunctionType

    def bcast_inner(ap, inner):
        """ap [128, k] -> [128, k, inner] with 0-stride inner axis."""
        return bass.AP(
            tensor=ap.tensor,
            offset=ap.offset,
            ap=[list(ap.ap[0]), list(ap.ap[1]), [0, inner]],
        )

    def bcast_rep(ap, reps):
        """ap [128, k] -> [128, reps, k] with 0-stride middle axis."""
        return bass.AP(
            tensor=ap.tensor,
            offset=ap.offset,
            ap=[list(ap.ap[0]), [0, reps], list(ap.ap[1])],
        )

    with tile.TileContext(nc) as tc:
        with (
            tc.tile_pool(name="consts", bufs=1) as consts,
            tc.tile_pool(name="gwin", bufs=6) as gwinp,
            tc.tile_pool(name="gov", bufs=3) as govp,
            tc.tile_pool(name="ohp", bufs=4) as ohp,
            tc.tile_pool(name="aggp", bufs=4) as aggp,
            tc.tile_pool(name="hraw", bufs=1) as hrawp,
            tc.tile_pool(name="statp", bufs=2) as statp,
            tc.tile_pool(name="small", bufs=8) as small,
            tc.tile_pool(name="p2", bufs=3) as p2p,
            tc.tile_pool(name="outp", bufs=1) as outp,
            tc.tile_pool(name="ps_agg", bufs=2, space="PSUM") as ps_agg,
            tc.tile_pool(name="ps_h", bufs=2, space="PSUM") as ps_h,
            tc.tile_pool(name="ps_t", bufs=2, space="PSUM") as ps_t,
        ):
            # ---- layer-invariant inputs (indices, dest_rel, weights, consts)
            wi_sb = consts.tile(list(shapes["win_idx"]), i16, tag="wi")
            nc.sync.dma_start(out=wi_sb[:], in_=wi_d[:])
            wd_sb = consts.tile(list(shapes["win_drel"]), f32, tag="wd")
            nc.sync.dma_start(out=wd_sb[:], in_=wd_d[:])
            oi_sb = consts.tile(list(shapes["ov_idx"]), i16, tag="oi")
            nc.sync.dma_start(out=oi_sb[:], in_=oi_d[:])
            od_sb = consts.tile(list(shapes["ov_drel"]), f32, tag="od")
            nc.sync.dma_start(out=od_sb[:], in_=od_d[:])

            w_sb = []
            for wdr in (w1_d, w2_d):
                t = consts.tile([P, FEAT], bf16, tag=f"w_{wdr.name}")
                nc.sync.dma_start(out=t[:], in_=wdr[:])
                w_sb.append(t)
            w3_sb = consts.tile([P, 1], bf16, tag="w3")
            nc.sync.dma_start(out=w3_sb[:], in_=w3_d[:])
            b_sb = []
            for bd in (b1_d, b2_d):
                t = consts.tile([P, 1], f32, tag=f"b_{bd.name}")
                nc.sync.dma_start(out=t[:], in_=bd[:, None])
                b_sb.append(t)
            b3_sb = consts.tile([P, 1], f32, tag="b3")
            nc.sync.dma_start(out=b3_sb[:], in_=b3_d[:].to_broadcast([P, 1]))
            gb_sb = []
            for gd, bd in ((gam1_d, bet1_d), (gam2_d, bet2_d)):
                tg_ = consts.tile([P, 1], f32, tag=f"g_{gd.name}")
                nc.sync.dma_start(out=tg_[:], in_=gd[:, None])
                tb_ = consts.tile([P, 1], f32, tag=f"be_{bd.name}")
                nc.sync.dma_start(out=tb_[:], in_=bd[:, None])
                gb_sb.append((tg_, tb_))
            iota_w = consts.tile([P, WIN], f32, tag="iota_w")
            nc.sync.dma_start(out=iota_w[:], in_=iota_w_d[:])
            iota_p = consts.tile([P, P], f32, tag="iota_p")
            nc.sync.dma_start(out=iota_p[:], in_=iota_p_d[:])
            ident = consts.tile([P, P], bf16, tag="ident")
            nc.sync.dma_start(out=ident[:], in_=ident_d[:])
            eps_sb = consts.tile([P, 1], f32, tag="eps")
            nc.vector.memset(eps_sb[:], cfg.EPS)

            for layer in range(cfg.LAYERS):
                table = xm_d if layer == 0 else htab[layer - 1]
                is_last = layer == cfg.LAYERS - 1
                if not is_last:
                    # hraw: [fout, dest] per batch, f32
                    hraw = [
                        hrawp.tile(
                            [P, cfg.TILES * P], f32, tag=f"hraw{bat}",
                            name=f"hraw{bat}",
                        )
                        for bat in range(2)
                    ]
                    stat_t = statp.tile([P, 2 * cfg.TILES, 6], f32, tag="stats")
                else:
                    out_sb = [
                        outp.tile(
                            [P, cfg.TILES], f32, tag=f"outsb{bat}",
                            name=f"outsb{bat}",
                        )
                        for bat in range(2)
                    ]

                for og in range(NOVG):
                    t0, t1 = og * OG, min((og + 1) * OG, cfg.TILES)
                    ogg = t1 - t0
                    gov = []
                    if OVC:
                        nch = ogg * OVC
                        base_i = og * OG * OVC * 8 * 2
                        for b in (0, 1):
                            gt = govp.tile([P, OG * OVC, TW], bf16, tag=f"gov{b}")
                            src = table[:, :] if b == 0 else table[cfg.HALF :, :]
                            nc.gpsimd.dma_gather(
                                gt[:, :nch, :],
                                src,
                                oi_sb[:, base_i + b * nch * 8 : base_i + (b + 1) * nch * 8],
                                nch * P,
                                nch * P,
                                TW,
                            )
                            gov.append(gt)

                    for t in range(t0, t1):
                        tl = t - t0
                        gwin = []
                        for b in (0, 1):
                            gt = gwinp.tile([P, WPT, TW], bf16, tag=f"gwin{b}")
                            src = table[:, :] if b == 0 else table[cfg.HALF :, :]
                            nc.gpsimd.dma_gather(
                                gt[:],
                                src,
                                wi_sb[:, t * P + b * 64 : t * P + (b + 1) * 64],
                                WPT * P,
                                WPT * P,
                                TW,
                            )
                            gwin.append(gt)
                        oh_w = []
                        oh_o = []
                        for b in (0, 1):
                            t_ohw = ohp.tile([P, WPT * WIN], bf16, tag=f"ohw{b}")
                            nc.vector.tensor_tensor(
                                out=t_ohw[:],
                                in0=bcast_inner(
                                    wd_sb[:, t * WIN + b * WPT : t * WIN + (b + 1) * WPT],
                                    WIN,
                                ),
                                in1=bcast_rep(iota_w[:], WPT),
                                op=AluOp.is_equal,
                            )
                            oh_w.append(t_ohw)
                            if OVC:
                                base_d = og * OG * OVC * 2
                                c0 = base_d + b * ogg * OVC + tl * OVC
                                t_oho = ohp.tile([P, OVC * P], bf16, tag=f"oho{b}")
                                nc.vector.tensor_tensor(
                                    out=t_oho[:],
                                    in0=bcast_inner(od_sb[:, c0 : c0 + OVC], P),
                                    in1=bcast_rep(iota_p[:], OVC),
                                    op=AluOp.is_equal,
                                )
                                oh_o.append(t_oho)

                        # two PSUM accumulators, one per batch element
                        agg_ps = [
                            ps_agg.tile(
                                [P, P], f32, tag=f"agg{bat}", name=f"agg{bat}"
                            )
                            for bat in range(2)
                        ]
                        n_ov = 2 * OVC
                        for bat in range(2):
                            fsl = slice(bat * FEAT, (bat + 1) * FEAT)
                            for b in (0, 1):
                                for w in range(WPT):
                                    nc.tensor.matmul(
                                        agg_ps[bat][:, w * WIN : (w + 1) * WIN],
                                        lhsT=gwin[b][:, w, fsl],
                                        rhs=oh_w[b][:, w * WIN : (w + 1) * WIN],
                                        start=(b == 0 and w == 0),
                                        stop=(n_ov == 0 and b == 1 and w == WPT - 1),
                                    )
                            k_ov = 0
                            for b in range(2):
                                for j in range(OVC):
                                    k_ov += 1
                                    nc.tensor.matmul(
                                        agg_ps[bat][:, :],
                                        lhsT=gov[b][:, tl * OVC + j, fsl],
                                        rhs=oh_o[b][:, j * P : (j + 1) * P],
                                        start=False,
                                        stop=(k_ov == n_ov),
                                    )

                        valid = cfg.VALID_LAST if t == cfg.TILES - 1 else P
                        for bat in range(2):
                            agg_sb = aggp.tile([P, P], bf16, tag=f"aggsb{bat}")
                            nc.vector.tensor_copy(out=agg_sb[:], in_=agg_ps[bat][:])
                            if not is_last:
                                h_ps = ps_h.tile([P, P], f32, tag="hps")
                                nc.tensor.matmul(
                                    h_ps[:], lhsT=w_sb[layer][:], rhs=agg_sb[:],
                                    start=True, stop=True,
                                )
                                nc.vector.tensor_scalar_add(
                                    out=hraw[bat][:, t * P : t * P + P],
                                    in0=h_ps[:],
                                    scalar1=b_sb[layer][:],
                                )
                                nc.vector.bn_stats(
                                    out=stat_t[:, 2 * t + bat, :],
                                    in_=hraw[bat][:, t * P : t * P + valid],
                                )
                            else:
                                o_ps = ps_h.tile([P, 1], f32, tag="hps")
                                nc.tensor.matmul(
                                    o_ps[:], lhsT=agg_sb[:], rhs=w3_sb[:],
                                    start=True, stop=True,
                                )
                                nc.vector.tensor_scalar_add(
                                    out=out_sb[bat][:, t : t + 1],
                                    in0=o_ps[:],
                                    scalar1=b3_sb[:],
                                )

                if not is_last:
                    # ---- global BN stats
                    mv = small.tile([P, 2], f32, tag="mv")
                    nc.vector.bn_aggr(out=mv[:], in_=stat_t[:, :, :])
                    sloc = small.tile([P, 2], f32, tag="sloc")
                    nc.vector.tensor_copy(out=sloc[:, 0:1], in_=mv[:, 0:1])
                    nc.vector.tensor_tensor(
                        out=sloc[:, 1:2], in0=mv[:, 0:1], in1=mv[:, 0:1], op=AluOp.mult
                    )
                    nc.vector.tensor_add(
                        out=sloc[:, 1:2], in0=sloc[:, 1:2], in1=mv[:, 1:2]
                    )
                    nc.sync.dma_start(out=stat_in[layer][:], in_=sloc[:])
                    if cfg.USE_AR:
                        nc.gpsimd.collective_compute(
                            "AllReduce",
                            AluOp.add,
                            replica_groups=[[0, 1, 2, 3, 4, 5, 6, 7]],
                            ins=[stat_in[layer][:]],
                            outs=[stat_out[layer][:]],
                        )
                    else:
                        nc.sync.dma_start(out=stat_out[layer][:], in_=stat_in[layer][:])
                    sglob = small.tile([P, 2], f32, tag="sglob")
                    nc.sync.dma_start(out=sglob[:], in_=stat_out[layer][:])
                    nc.scalar.mul(
                        out=sglob[:], in_=sglob[:], mul=0.125 if cfg.USE_AR else 1.0
                    )
                    var = small.tile([P, 1], f32, tag="var")
                    nc.vector.tensor_tensor(
                        out=var[:], in0=sglob[:, 0:1], in1=sglob[:, 0:1], op=AluOp.mult
                    )
                    nc.vector.tensor_sub(out=var[:], in0=sglob[:, 1:2], in1=var[:])
                    rstd = small.tile([P, 1], f32, tag="rstd")
                    nc.scalar.activation(
                        out=rstd[:], in_=var[:], func=ActF.Sqrt, bias=eps_sb[:]
                    )
                    nc.vector.reciprocal(out=rstd[:], in_=rstd[:])
                    scal = small.tile([P, 1], f32, tag="scal")
                    nc.vector.tensor_tensor(
                        out=scal[:], in0=gb_sb[layer][0][:], in1=rstd[:], op=AluOp.mult
                    )
                    shif = small.tile([P, 1], f32, tag="shif")
                    nc.vector.tensor_tensor(
                        out=shif[:], in0=sglob[:, 0:1], in1=scal[:], op=AluOp.mult
                    )
                    nc.vector.tensor_sub(out=shif[:], in0=gb_sb[layer][1][:], in1=shif[:])
                    # ---- pass 2: BN + relu + transpose + write shard rows
                    for t in range(cfg.TILES):
                        valid = cfg.VALID_LAST if t == cfg.TILES - 1 else P
                        for bat in range(2):
                            hbn = p2p.tile([P, P], bf16, tag=f"hbn{bat}")
                            nc.scalar.activation(
                                out=hbn[:],
                                in_=hraw[bat][:, t * P : (t + 1) * P],
                                func=ActF.Relu,
                                bias=shif[:],
                                scale=scal[:],
                            )
                            t_ps = ps_t.tile([P, P], bf16, tag="tps")
                            nc.tensor.transpose(
                                out=t_ps[:], in_=hbn[:], identity=ident[:]
                            )
                            hrow = p2p.tile([P, P], bf16, tag=f"hrow{bat}")
                            nc.vector.tensor_copy(out=hrow[:], in_=t_ps[:])
                            nc.sync.dma_start(
                                out=shard_out[layer][
                                    t * P : t * P + valid,
                                    bat * FEAT : (bat + 1) * FEAT,
                                ],
                                in_=hrow[:valid, :],
                            )
                    if cfg.USE_AG:
                        nc.gpsimd.collective_compute(
                            "AllGather",
                            AluOp.bypass,
                            replica_groups=[[0, 1, 2, 3, 4, 5, 6, 7]],
                            ins=[shard_out[layer][:]],
                            outs=[htab[layer][:]],
                        )
                    else:
                        nc.sync.dma_start(
                            out=htab[layer][0 : cfg.SHARD, :], in_=shard_out[layer][:]
                        )
                else:
                    nfull = cfg.TILES - 1
                    for bat, od_ in ((0, out0_d), (1, out1_d)):
                        if nfull:
                            nc.sync.dma_start(
                                out=od_[0 : nfull * P].rearrange("(t p) -> p t", p=P),
                                in_=out_sb[bat][:, 0:nfull],
                            )
                        nc.sync.dma_start(
                            out=od_[nfull * P : cfg.SHARD, None],
                            in_=out_sb[bat][: cfg.VALID_LAST, nfull : nfull + 1],
                        )

    nc.compile()
    return nc


# ---------------------------------------------------------------- consts + run


def _const_inputs():
    import jax.numpy as jnp

    iota_w = np.tile(np.arange(WIN, dtype=np.float32), (P, 1))
    iota_p = np.tile(np.arange(P, dtype=np.float32), (P, 1))
    ident = np.asarray(jnp.asarray(np.eye(P, dtype=np.float32), dtype=jnp.float16))
    return iota_w, iota_p, ident


def run_gcn(cfg, inputs, trace=False):
    import jax.numpy as jnp

    from concourse.bass_utils import run_bass_kernel_spmd

    def bf(a):
        return np.asarray(jnp.asarray(np.asarray(a, np.float32), dtype=jnp.float16))

    x = np.asarray(inputs["x"], dtype=np.float32)
    edge_index = np.asarray(inputs["edge_index"])
    per_shard, OVC, OG = build_schedule(cfg, edge_index)
    shapes = {k: v.shape for k, v in per_shard[0].items()}
    nc = build_nc(cfg, OVC, OG, shapes)

    # merged bf16 node table: row n = [x0[n](128) | x1[n](128)]
    xm = np.concatenate([x[0], x[1]], axis=1)
    xm = bf(xm)

    iota_w, iota_p, ident = _const_inputs()
    common = {
        "xm_tab": xm,
        "W1": bf(inputs["W1"]),
        "W2": bf(inputs["W2"]),
        "W3": bf(inputs["W3"]),
        "b1": np.asarray(inputs["b1"], np.float32),
        "b2": np.asarray(inputs["b2"], np.float32),
        "b3": np.asarray(inputs["b3"], np.float32),
        "gamma1": np.asarray(inputs["gamma1"], np.float32),
        "beta1": np.asarray(inputs["beta1"], np.float32),
        "gamma2": np.asarray(inputs["gamma2"], np.float32),
        "beta2": np.asarray(inputs["beta2"], np.float32),
        "iota_w": iota_w,
        "iota_p": iota_p,
        "ident": ident,
    }
    in_maps = []
    for c in range(NSHARD):
        m = dict(common)
        m.update(per_shard[c])
        in_maps.append(m)

    try:
        res = run_bass_kernel_spmd(nc, in_maps, core_ids=list(range(8)), trace=trace)
    except ModuleNotFoundError:
        res = run_bass_kernel_spmd(nc, in_maps, core_ids=list(range(8)), trace=False)
    out = np.empty((cfg.BATCH, cfg.N), np.float32)
    for c in range(NSHARD):
        out[0, c * cfg.SHARD : (c + 1) * cfg.SHARD] = res.results[c]["out0"]
        out[1, c * cfg.SHARD : (c + 1) * cfg.SHARD] = res.results[c]["out1"]
    return out, res


def kernel(**inputs) -> np.ndarray:
    cfg = Cfg()
    out, _ = run_gcn(cfg, inputs, trace=False)
    return out
